# revision 113
# baseline (speedup 1.0000x reference)
"""Decoder block kernel for 8 Trainium2 NeuronCores.

Sharding: core = 2*b + h handles batch b, query tokens q with q % 2 == h
(interleaved so the causal-mask block structure is identical on every
core -> one SPMD program; the mask diagonal band differs only in DATA).

All activations live transposed [C, tokens] (C on partitions), so every
linear layer uses the stored [in,out] weights directly as the stationary
operand and no on-device transposes are needed. LayerNorm statistics are
computed with ones-matmuls on the PE (replicated across partitions);
softmax row sums come from a ones-column appended to V.

v2: matmul operands and streamed tensors are bfloat16 (weights pre-tiled
on the host so every DMA moves >=512B contiguous runs); LN/softmax stat
chains, folds and the final output stay fp32. Causal score/AV ranges are
trimmed to exact 64-column boundaries (no fp32r >=256-column constraint
with bf16 moving operands). Scalar constants ship as one packed block.
"""
import numpy as np

B, T, C, H, D, FF = 4, 1024, 1024, 16, 64, 4096
NT = C // 128   # 8 partition tiles of the model dim
KT = T // 128   # 8 context-token tiles
FT = FF // 128  # 32
TQ = T // 2     # 512 local query tokens per core

# packed fp32 const block: name -> (col0, ncols)
COLS = {"pad1": (0, 8), "pad2": (8, 8), "g1": (16, 8), "b1": (24, 8),
        "g2": (32, 8), "b2": (40, 8), "g3": (48, 8), "b3": (56, 8),
        "bf1": (64, 32), "csq2": (96, 8), "bq2": (104, 8),
        "csf1": (112, 32), "bf2": (144, 8), "tri": (152, 32)}
NCOL = 184

_CACHE = {}


def _build(repeat=1):
    import concourse.bacc as bacc
    import concourse.tile as tile
    from concourse import mybir

    # The act-table placement pass assigns each Activation the FIRST table
    # set containing its function. This kernel only uses Exp/Ln/Identity/
    # Relu/Copy, which coexist in the natural_log_exp_and_others set —
    # hiding those funcs from the OTHER sets (keeping list order, since
    # act_func_set_id is positional in act_info.json) makes the whole
    # kernel run off that one table: no 1.3us LoadActFuncSet reloads
    # between softmax Exp and the LN chains.
    _orig_tables = bacc.get_activation_tables

    def _tables_ln_exp_only(arch):
        tabs = _orig_tables(arch)
        key = "natural_log_exp_and_others"
        if key not in tabs:
            return tabs
        mine = {f for f in tabs[key]
                if f.name in ("Exp", "Ln", "Identity", "Relu", "Copy",
                              "Square")}
        return {k: (v if k == key else v - mine) for k, v in tabs.items()}

    bacc.get_activation_tables = _tables_ln_exp_only
    try:
        nc = _build_inner(bacc, tile, mybir, repeat)
    finally:
        bacc.get_activation_tables = _orig_tables
    return nc


def _build_inner(bacc, tile, mybir, repeat):
    nc = bacc.Bacc(None, target_bir_lowering=False)
    F32 = mybir.dt.float32
    BF16 = mybir.dt.bfloat16

    def din(name, shape, dt=BF16):
        return nc.dram_tensor(name, shape, dt, kind="ExternalInput")

    t = {}
    t["xT"] = din("xT", [C, T])
    t["xTl"] = din("xTl", [C, TQ])
    t["encT"] = din("encT", [C, T])
    # pre-tiled [ot, p, ct, o] layouts
    for k in ("wq1", "wk1", "wo1", "wq2", "wk2", "wo2"):
        t[k] = din(k, [C, C])
    t["wf1"] = din("wf1", [C, FF])        # pre-scaled by diag(g2), pretiled
    t["wf2"] = din("wf2", [FF, C])        # pretiled
    # natural [in, out] layouts (moving operand of the V matmul)
    t["wv1"] = din("wv1", [C, C])
    t["wv2"] = din("wv2", [C, C])
    t["cblk"] = din("cblk", [128, NCOL], F32)
    t["outT"] = nc.dram_tensor("outT", [C, TQ], F32, kind="ExternalOutput")

    with tile.TileContext(nc) as tc:
        for it in range(repeat):
            _emit(nc, tc, t, it)
    nc.compile()
    return nc



def _emit(nc, tc, t, it):
    from contextlib import ExitStack
    import concourse.bass as bass
    from concourse import mybir
    from concourse.tile import add_dep_helper

    F32 = mybir.dt.float32
    F32R = mybir.dt.float32r
    BF16 = mybir.dt.bfloat16
    AF = mybir.ActivationFunctionType
    ALU = mybir.AluOpType

    def w_ap(wdram, nctt, ot, a0, na):
        """pretiled arr[ot, p, ct, o]: [128, na, 128] view at (ot, a0)"""
        return bass.AP(tensor=wdram, offset=ot * nctt * 128 * 128 + a0 * 128,
                       ap=[[nctt * 128, 128], [128, na], [1, 128]])

    with ExitStack() as ctx:
        consts = ctx.enter_context(tc.tile_pool(name=f"con{it}", bufs=1))
        cb = consts.tile([128, NCOL], F32, tag="cb", name="cb")
        cb_dma = [None]

        def emit_cb(after=None):
            di = nc.scalar.dma_start(cb[:], t["cblk"][:])
            if after is not None:
                add_dep_helper(di.ins, after.ins,
                               reason="consts after critical startup stream")
            cb_dma[0] = di
        tri_sb = cb[:, COLS["tri"][0]:COLS["tri"][0] + 32].bitcast(BF16)
        ones128 = consts.tile([128, 128], BF16, tag="o128", name="o128")
        nc.vector.memset(ones128[:], 1.0)
        ones1 = consts.tile([1, 128], F32R, tag="o1", name="o1")
        nc.vector.memset(ones1[:].bitcast(F32), 1.0)
        eps_t = consts.tile([128, 1], F32, tag="eps", name="eps")
        nc.vector.memset(eps_t[:], 1e-5)

        lv = {k: cb[:, c0:c0 + n] for k, (c0, n) in COLS.items()}
        pad_sb = {"pad1": lv["pad1"], "pad2": lv["pad2"]}

        wpool = ctx.enter_context(tc.tile_pool(name=f"wp{it}", bufs=8))
        ypool = ctx.enter_context(tc.tile_pool(name=f"yp{it}", bufs=1))
        foldp = ctx.enter_context(tc.tile_pool(name=f"fp{it}", bufs=3))

        def fold_epilogue(ps, fold, ot, out_tile, func, bias_sb):
            """out = func(rstd*(ps - m*CS[ot]) + bias) given fold=(rstd, un)
            with un = -m*rstd, CS per-output-channel colsum."""
            rstd, un, cs = fold
            # PSUM reads and TensorScalarPtr are DVE/Act-only ops; GPSIMD
            # handles neither, so the whole fold chain stays on DVE.
            ftile = foldp.tile([128, TQ], F32, tag="ft", name="ft")
            nc.vector.tensor_mul(ftile[:], ps[:], rstd[:])
            nc.vector.scalar_tensor_tensor(
                out=ftile[:], in0=un[:], scalar=cs[:, ot:ot + 1], in1=ftile[:],
                op0=ALU.mult, op1=ALU.add)
            with nc.allow_low_precision(reason="bf16 activations"):
                nc.scalar.activation(out_tile[:], ftile[:], func,
                                     bias=bias_sb[:, ot:ot + 1])

        def new_y(dt=BF16):
            return [ypool.tile([128, TQ], dt, tag=f"y{i}", name=f"y{i}")
                    for i in range(NT)]

        def linear_T(wdram, cin, cout, in_tiles, n, epilogue, pp,
                     dma_out=None, first_cb=None):
            """psum[ot][:, q0:] = sum_ct W[ct,ot].T @ in[ct][:, q0:]"""
            nct = cin // 128
            for ot in range(cout // 128):
                wt = wpool.tile([128, nct, 128], BF16, tag="w", name="w")
                di = nc.sync.dma_start(wt[:], w_ap(wdram, nct, ot, 0, nct))
                if dma_out is not None:
                    dma_out.append(di)
                if ot == 0 and first_cb is not None:
                    first_cb()
                for q0 in range(0, n, 512):
                    ps = pp.tile([128, 512], F32, tag="pp", name="pp")
                    for ct in range(nct):
                        nc.tensor.matmul(ps[:], wt[:, ct, :],
                                         in_tiles[ct][:, q0:q0 + 512],
                                         start=(ct == 0), stop=(ct == nct - 1))
                    epilogue(ot, q0, ps)

        def ln_begin(sctx, tagp, lnps, lntag):
            sqp = sctx.enter_context(tc.tile_pool(name=f"sq{tagp}{it}", bufs=2))
            scr = sctx.enter_context(tc.tile_pool(name=f"ls{tagp}{it}", bufs=1))
            ps1 = lnps.tile([128, TQ], F32, tag=lntag, name="ln")
            ps2 = lnps.tile([128, TQ], F32, tag=lntag, name="ln")
            return {"sqp": sqp, "scr": scr, "ps1": ps1, "ps2": ps2}

        def ln_feed(st, y_tile, ct, sl=slice(0, TQ), sq_eng=None):
            """Accumulate sum(y) and sum(y^2) for one partition tile."""
            w = sl.stop - sl.start
            sq = st["sqp"].tile([128, TQ], BF16, tag="sq", name="sq")
            with nc.allow_low_precision(reason="bf16 squares ok"):
                (sq_eng or nc.vector).tensor_mul(sq[:, 0:w], y_tile[:, sl],
                                                 y_tile[:, sl])
            nc.tensor.matmul(st["ps1"][:, sl], ones128[:], y_tile[:, sl],
                             start=(ct == 0), stop=(ct == NT - 1),
                             skip_group_check=True)
            nc.tensor.matmul(st["ps2"][:, sl], ones128[:], sq[:, 0:w],
                             start=(ct == 0), stop=(ct == NT - 1),
                             skip_group_check=True)

        def ln_finish(st, y_in, g, b, out_t, statpool):
            """Stats chain now; the per-tile normalize (overwrites y_in) is
            returned as a closure so the caller can emit it where the vector
            engines are idle. Returns ((rstd, un), emit_norm)."""
            sp_ = statpool
            m = sp_.tile([128, TQ], F32, tag="m", name="m")
            nc.scalar.mul(m[:], st["ps1"][:], 1.0 / C)
            m2 = sp_.tile([128, TQ], F32, tag="v2", name="v2")
            nc.scalar.activation(m2[:], m[:], AF.Square)
            ms = sp_.tile([128, TQ], F32, tag="v", name="v")
            nc.vector.tensor_scalar_mul(ms[:], st["ps2"][:], 1.0 / C)
            nc.vector.tensor_sub(ms[:], ms[:], m2[:])
            # rstd = (v+eps)^-0.5 = exp(-0.5*ln(v+eps)): Ln+Exp live in the
            # same activation-table set as attention's Exp, so the Act engine
            # never reloads tables (Sqrt would force a 1.3us switch).
            nc.scalar.activation(ms[:], ms[:], AF.Ln, bias=eps_t[:])
            rstd = sp_.tile([128, TQ], F32, tag="r", name="r")
            nc.scalar.activation(rstd[:], ms[:], AF.Exp, scale=-0.5)
            un = sp_.tile([128, TQ], F32, tag="un", name="un")
            nc.vector.scalar_tensor_tensor(
                out=un[:], in0=m[:], scalar=-1.0, in1=rstd[:],
                op0=ALU.mult, op1=ALU.mult)

            def emit_norm(cts):
                for ct in cts:
                    eng = nc.gpsimd if ct in (3, 7) else nc.vector
                    d = foldp.tile([128, TQ], F32, tag="dn", name="dn",
                                   bufs=4)
                    eng.tensor_sub(d[:], y_in[ct][:], m[:])
                    eng.tensor_mul(d[:], d[:], rstd[:])
                    with nc.allow_low_precision(reason="bf16 activations"):
                        nc.scalar.activation(out_t[ct][:], d[:], AF.Identity,
                                             bias=b[:, ct:ct + 1],
                                             scale=g[:, ct:ct + 1])
            return (rstd, un), emit_norm

        def ln_chunk(st, y_in, g, b, y3t, sl):
            """Finish one column chunk: stats chain + normalize + out DMA."""
            scr = st["scr"]
            w = sl.stop - sl.start
            if "m" not in st:
                st["m"] = scr.tile([128, TQ], F32, tag="m", name="m", bufs=1)
                st["v"] = scr.tile([128, TQ], F32, tag="v", name="v", bufs=1)
                st["v2"] = scr.tile([128, TQ], F32, tag="v2", name="v2",
                                    bufs=1)
                st["r"] = scr.tile([128, TQ], F32, tag="r", name="r", bufs=1)
            m, ms, m2, rstd = st["m"], st["v"], st["v2"], st["r"]
            # m via Act (PSUM-read ok) in parallel with ms on DVE, then the
            # square also on Act back-to-back — one less cross-engine hop
            nc.scalar.mul(m[:, sl], st["ps1"][:, sl], 1.0 / C)
            nc.scalar.activation(m2[:, sl], m[:, sl], AF.Square)
            nc.vector.tensor_scalar_mul(ms[:, sl], st["ps2"][:, sl], 1.0 / C)
            nc.vector.tensor_sub(ms[:, sl], ms[:, sl], m2[:, sl])
            nc.scalar.activation(ms[:, sl], ms[:, sl], AF.Ln, bias=eps_t[:])
            nc.scalar.activation(rstd[:, sl], ms[:, sl], AF.Exp, scale=-0.5)
            last = True
            for ct in range(NT):
                eng = nc.vector if ct % 2 == 0 else nc.gpsimd
                d = scr.tile([128, w], F32, tag="dc", name="dc", bufs=6)
                eng.tensor_sub(d[:], y_in[ct][:, sl], m[:, sl])
                eng.tensor_mul(d[:], d[:], rstd[:, sl])
                nc.scalar.activation(y3t[:, ct, sl], d[:], AF.Identity,
                                     bias=b[:, ct:ct + 1],
                                     scale=g[:, ct:ct + 1])
                if last and ct == 3:
                    nc.sync.dma_start(
                        bass.AP(tensor=t["outT"], offset=sl.start,
                                ap=[[TQ, 128], [128 * TQ, 4], [1, w]]),
                        y3t[:, 0:4, sl])
            if last:
                nc.sync.dma_start(
                    bass.AP(tensor=t["outT"], offset=4 * 128 * TQ + sl.start,
                            ap=[[TQ, 128], [128 * TQ, 4], [1, w]]),
                    y3t[:, 4:8, sl])
            else:
                nc.sync.dma_start(
                    bass.AP(tensor=t["outT"], offset=sl.start,
                            ap=[[TQ, 128], [128 * TQ, NT], [1, w]]),
                    y3t[:, :, sl])

        def kv_stage(src, wk_d, wv_d, wq_d, q_src, padkey, kTt, vtt,
                     qTt, tagp, q_first, pp, qfold=None, src_dma=None,
                     after_k=None, mid_hook=None, q_first_cb=None):
            """Compute K_T, V (pad-masked, with the pad column in slot 64 for
            the softmax row sums), and Q_T from a resident transposed source.
            q_first=True: emit Q's matmuls before K (source already resident,
            weight DMAs returned through src_dma ordering); False: K first so
            the PE can run while the previous LN chain finishes."""
            with ExitStack() as kctx:
                def emit_q(dma_out=None):
                    if qfold is None:
                        def ep(ot, q0, ps):
                            with nc.allow_low_precision(reason="bf16 acts"):
                                nc.scalar.copy(qTt[ot][:], ps[:])
                        linear_T(wq_d, C, C, q_src, TQ, ep, pp,
                                 dma_out=dma_out, first_cb=q_first_cb)
                    else:
                        linear_T(wq_d, C, C, q_src, TQ,
                                 lambda ot, q0, ps: fold_epilogue(
                                     ps, qfold, ot, qTt[ot], AF.Identity,
                                     lv["bq2"]), pp, dma_out=dma_out)

                if q_first:
                    wd = []
                    emit_q(dma_out=wd)
                    if src_dma is not None:
                        src_dma(wd)

                def kep(ot, q0, ps):
                    # PSUM source: only DVE/Act may read PSUM (not GPSIMD)
                    with nc.allow_low_precision(reason="bf16 acts"):
                        nc.vector.tensor_copy(kTt[ot][:, q0:q0 + 512], ps[:])
                kd = []
                linear_T(wk_d, C, C, src, T, kep, pp, dma_out=kd)
                if after_k is not None:
                    after_k(kd)
                if not q_first:
                    emit_q()
                if mid_hook is not None:
                    mid_hook()
                wvp = kctx.enter_context(tc.tile_pool(name=f"wv{tagp}{it}",
                                                      bufs=2))
                for tt in range(KT):
                    with nc.allow_low_precision(reason="bf16 pad col"):
                        nc.gpsimd.tensor_scalar_mul(
                            vtt[tt][:, :, 64:65], ones128[:, 0:16],
                            pad_sb[padkey][:, tt:tt + 1])
                for half in range(2):
                    wvq = []
                    for cq in range(4):
                        wvt = wvp.tile([128, 2, 512], BF16, tag="wv",
                                       name="wv", bufs=8)
                        nc.sync.dma_start(
                            wvt[:],
                            bass.AP(tensor=wv_d,
                                    offset=512 * half + 256 * cq * C,
                                    ap=[[C, 128], [128 * C, 2], [1, 512]]))
                        wvq.append(wvt)
                    for tt in range(KT):
                        ps = pp.tile([128, 512], F32, tag="pp", name="pp")
                        for ct in range(NT):
                            nc.tensor.matmul(
                                ps[:], src[ct][:, 128 * tt:128 * (tt + 1)],
                                wvq[ct // 2][:, ct % 2, :],
                                start=(ct == 0), stop=(ct == NT - 1))
                        with nc.allow_low_precision(reason="bf16 acts"):
                            nc.vector.tensor_scalar_mul(
                                vtt[tt][:, 8 * half:8 * (half + 1), 0:64],
                                ps[:].rearrange("p (h d) -> p h d", d=64),
                                pad_sb[padkey][:, tt:tt + 1])

        def attention(qTt, kTt, vtt, wo_d, resid, g, b, out_t, causal,
                      sctx, tagp, scp, avp, rbp, op, statpool,
                      paired=False):
            """Scores/AV in transposed layout; O-projection output (+resid)
            is written back into the qT tiles (dead by then), then LN."""
            with ExitStack() as atx:
                pvp = atx.enter_context(tc.tile_pool(
                    name=f"pv{tagp}{it}", bufs=1))
                ppool = atx.enter_context(tc.tile_pool(
                    name=f"pt{tagp}{it}", bufs=8))
                sbp = atx.enter_context(tc.tile_pool(
                    name=f"sb{tagp}{it}", bufs=2))
                pv = [pvp.tile([128, TQ], BF16, tag=f"pv{i}", name=f"pv{i}")
                      for i in range(NT)]
                hseq = [(h0 + 4) % H for h0 in range(H)]
                for hpos, h in enumerate(hseq):
                    ct, off = h // 2, (h % 2) * 64
                    av = avp.tile([65, 512], F32, tag="av", name="av")
                    pend = None

                    def emit_av(p, c, k, av=av, h=h):
                        nc.tensor.matmul(av[:, c:512], vtt[k][:, h, 0:65],
                                         p[:, c:512],
                                         start=(k == 0), stop=(k == KT - 1))

                    if paired:
                        # score two key tiles into one 2-bank PSUM tile and
                        # exponentiate both with ONE Act instruction (Act
                        # per-instr overhead pins the attention windows).
                        # Causal pairs (k+4, k): the longer-range tile sits
                        # first so one exp range [64(k+4), 1024) covers both
                        # valid regions (the unwritten [512, 512+64k) sliver
                        # is exp'd but never read). AV for kt=0 is emitted
                        # first within its pair: its full-span matmul carries
                        # the PSUM-initializing start flag.
                        pairs = ([(k + 4, k) for k in range(4)] if causal
                                 else [(2 * pk, 2 * pk + 1)
                                       for pk in range(4)])

                        def emit_pair_avs(Pt2, ka, kb):
                            order = ((1, kb), (0, ka)) if causal                                 else ((0, ka), (1, kb))
                            for j, kt in order:
                                emit_av(Pt2[:, 512 * j:512 * (j + 1)],
                                        64 * kt if causal else 0, kt)

                        for ka, kb in pairs:
                            sp2 = scp.tile([128, 1024], F32, tag="sc",
                                           name="sc")
                            Pt2 = ppool.tile([128, 1024], BF16, tag="P",
                                             name="P")
                            for j, kt in ((0, ka), (1, kb)):
                                c0 = 64 * kt if causal else 0
                                nc.tensor.matmul(
                                    sp2[:, 512 * j + c0:512 * (j + 1)],
                                    kTt[ct][off:off + 64,
                                            128 * kt:128 * (kt + 1)],
                                    qTt[ct][off:off + 64, c0:512],
                                    start=True, stop=True)
                            e0 = 64 * ka if causal else 0
                            with nc.allow_low_precision(reason="bf16 probs"):
                                nc.scalar.activation(Pt2[:, e0:1024],
                                                     sp2[:, e0:1024],
                                                     AF.Exp, scale=0.125)
                            if causal:
                                for j, kt in ((0, ka), (1, kb)):
                                    d0 = 512 * j + 64 * kt
                                    nc.gpsimd.tensor_mul(
                                        Pt2[:, d0:d0 + 64],
                                        Pt2[:, d0:d0 + 64], tri_sb[:])
                            if pend is not None:
                                emit_pair_avs(*pend)
                            pend = (Pt2, ka, kb)
                        emit_pair_avs(*pend)
                    else:
                        for kt in range(KT):
                            c0 = 64 * kt if causal else 0
                            sp = scp.tile([128, 512], F32, tag="pp",
                                          name="pp")
                            Pt = ppool.tile([128, TQ], BF16, tag="P",
                                            name="P")
                            nc.tensor.matmul(
                                sp[:, c0:512],
                                kTt[ct][off:off + 64,
                                        128 * kt:128 * (kt + 1)],
                                qTt[ct][off:off + 64, c0:512],
                                start=True, stop=True)
                            with nc.allow_low_precision(reason="bf16 probs"):
                                nc.scalar.activation(Pt[:, c0:512],
                                                     sp[:, c0:512],
                                                     AF.Exp, scale=0.125)
                            if causal:
                                nc.gpsimd.tensor_mul(
                                    Pt[:, 64 * kt:64 * (kt + 1)],
                                    Pt[:, 64 * kt:64 * (kt + 1)], tri_sb[:])
                            if pend is not None:
                                emit_av(*pend)
                            pend = (Pt, c0, kt)
                        emit_av(*pend)
                    rinv = sbp.tile([1, 512], F32R, tag="ri", name="ri")
                    with nc.allow_low_precision(reason="fp32r rounding ok"):
                        nc.vector.reciprocal(rinv[:], av[64:65, :])
                    if rbp is not None:
                        rb_ps = rbp.tile([128, 512], F32, tag="rb",
                                         name="rb")
                    else:
                        rb_ps = op.tile([128, 512], F32, tag="o", name="o")
                    nc.tensor.matmul(rb_ps[:], ones1[:], rinv[:],
                                     start=True, stop=True)
                    if hpos >= H - 2:
                        # tail heads: lift AV out of PSUM on the (now idle)
                        # Act engine in parallel with the reciprocal, and
                        # multiply against the PSUM broadcast directly — one
                        # DVE hop shorter, so WO isn't held up.
                        avs = sbp.tile([64, 512], F32, tag="avs", name="avs")
                        nc.scalar.copy(avs[:], av[0:64, :])
                        with nc.allow_low_precision(reason="bf16 acts"):
                            nc.vector.tensor_mul(pv[ct][off:off + 64, :],
                                                 avs[:], rb_ps[0:64, :])
                    else:
                        rb = sbp.tile([64, 512], F32, tag="rs", name="rs")
                        nc.vector.tensor_copy(rb[:], rb_ps[0:64, :])
                        with nc.allow_low_precision(reason="bf16 acts"):
                            nc.vector.tensor_mul(pv[ct][off:off + 64, :],
                                                 av[0:64, :], rb[:])
                lnst = ln_begin(atx, tagp, avp, "av")
                for co in range(NT):
                    wt = wpool.tile([128, NT, 128], BF16, tag="w", name="w")
                    nc.sync.dma_start(wt[:], w_ap(wo_d, NT, co, 0, NT))
                    ps = op.tile([128, 512], F32, tag="o", name="o")
                    for i, cc in enumerate((c + 2) % NT for c in range(NT)):
                        nc.tensor.matmul(ps[:], wt[:, cc, :], pv[cc][:],
                                         start=(i == 0), stop=(i == NT - 1))
                    with nc.allow_low_precision(reason="bf16 acts"):
                        nc.vector.tensor_add(qTt[co][:], ps[:], resid[co][:])
                    ln_feed(lnst, qTt[co], co)
                return ln_finish(lnst, qTt, g, b, out_t, statpool)

        # ================= main flow =================
        qpool = ctx.enter_context(tc.tile_pool(name=f"qp{it}", bufs=1))
        statp = ctx.enter_context(tc.tile_pool(name=f"st{it}", bufs=1))

        def new_q(pfx="q"):
            return [qpool.tile([128, TQ], BF16, tag=f"{pfx}{i}",
                               name=f"{pfx}{i}") for i in range(NT)]

        with ExitStack() as actx:
            qkv = actx.enter_context(tc.tile_pool(name=f"qkv{it}", bufs=1))
            srcp = actx.enter_context(tc.tile_pool(name=f"sr{it}", bufs=1))
            p1 = actx.enter_context(ExitStack())
            pps = p1.enter_context(tc.tile_pool(
                name=f"ps{it}", bufs=3, space="PSUM"))
            scs = p1.enter_context(tc.tile_pool(
                name=f"ss{it}", bufs=3, space="PSUM"))
            avps = p1.enter_context(tc.tile_pool(
                name=f"as{it}", bufs=2, space="PSUM"))

            def new_kv():
                k = [qkv.tile([128, T], BF16, tag=f"k{i}", name=f"k{i}")
                     for i in range(NT)]
                v = [qkv.tile([128, 16, 65], BF16, tag=f"v{i}", name=f"v{i}")
                     for i in range(KT)]
                return k, v

            def load_src(dram, tag, eng, after=None):
                st_ = srcp.tile([128, NT, T], BF16, tag=tag, name=tag)
                dmas = []
                for th in range(2):
                    di = eng.dma_start(
                        st_[:, :, 512 * th:512 * (th + 1)],
                        bass.AP(tensor=dram, offset=512 * th,
                                ap=[[T, 128], [128 * T, NT], [1, 512]]))
                    if after is not None and th < len(after):
                        add_dep_helper(di.ins, after[th].ins,
                                       reason="src after critical weights")
                    dmas.append(di)
                return [st_[:, i, :] for i in range(NT)], dmas

            # ---- self-attention + AddNorm ----
            qT = new_q()
            kTt, vtt = new_kv()
            with ExitStack() as sctx:
                xt_t = qpool.tile([128, NT, TQ], BF16, tag="xt", name="xt")
                xTl_sb = [xt_t[:, i, :] for i in range(NT)]

                def xt_emit():
                    # emitted after the first wq1 tile DMA so the weight
                    # stream heads the startup DMA queue; first ct alone so
                    # the very first matmul can fire early
                    dis = []
                    for c0_, nc_ in ((0, 1), (1, 3), (4, 4)):
                        dis.append(nc.sync.dma_start(
                            xt_t[:, c0_:c0_ + nc_, :],
                            bass.AP(tensor=t["xTl"], offset=c0_ * 128 * TQ,
                                    ap=[[TQ, 128], [128 * TQ, nc_], [1, TQ]])))
                    emit_cb(after=dis[0])
                src1, src2 = [None], [None]

                def src_dma(wd):
                    src1[0] = load_src(t["xT"], "sA", nc.gpsimd,
                                       after=[wd[1], wd[3]])[0]

                def enc_dma(kd):
                    # prefetch encT on the gpsimd queue once the K-weight
                    # stream (critical for the current stage) is in flight
                    src2[0] = load_src(t["encT"], "sB", nc.gpsimd,
                                       after=[kd[3], kd[7]])[0]

                class _SrcProxy:
                    def __getitem__(self, i):
                        return src1[0][i]
                kv_stage(_SrcProxy(), t["wk1"], t["wv1"], t["wq1"], xTl_sb,
                         "pad1", kTt, vtt, qT, "s", True, pps,
                         src_dma=src_dma, after_k=enc_dma,
                         q_first_cb=xt_emit)
                y1 = new_y()
                fold1, norm1 = attention(qT, kTt, vtt, t["wo1"], xTl_sb,
                                         lv["g1"], lv["b1"], y1, True,
                                         sctx, "s", bigp, avps, None, ops,
                                         statpool=statp, paired=True)

            # ---- cross-attention + AddNorm (fresh tile generations) ----
            qT2 = new_q("x")
            kTt2, vtt2 = new_kv()
            with ExitStack() as cctx:
                kv_stage(src2[0], t["wk2"], t["wv2"], t["wq2"], qT,
                         "pad2", kTt2, vtt2, qT2, "c", False, pps,
                         qfold=(fold1[0], fold1[1], lv["csq2"]),
                         mid_hook=lambda: norm1(range(NT)))
                # release the self-stage PSUM pools so cross-attention can
                # afford 2-bank paired score tiles (no PE filler competes
                # for PSUM in this window)
                p1.close()
                sc2ps = cctx.enter_context(tc.tile_pool(
                    name=f"sc{it}", bufs=2, space="PSUM"))
                avps2 = cctx.enter_context(tc.tile_pool(
                    name=f"a2{it}", bufs=2, space="PSUM"))
                ops2 = cctx.enter_context(tc.tile_pool(
                    name=f"o2{it}", bufs=2, space="PSUM"))
                y2 = new_y()
                fold2, norm2 = attention(qT2, kTt2, vtt2, t["wo2"], y1,
                                         lv["g2"], lv["b2"], y2, False,
                                         cctx, "c", sc2ps, avps2, None,
                                         ops2, statpool=statp, paired=True)

        # ---- FFN + AddNorm ----
        with ExitStack() as fctx:
            y3p = fctx.enter_context(tc.tile_pool(name=f"y3{it}", bufs=1))
            y3t = y3p.tile([128, NT, TQ], F32, tag="z", name="z")
            lnps3 = fctx.enter_context(tc.tile_pool(
                name=f"l3{it}", bufs=2, space="PSUM"))
            lnst3 = ln_begin(fctx, "f", lnps3, "ln")
            ffold = (fold2[0], fold2[1], lv["csf1"])
            with ExitStack() as mctx:
                hp = mctx.enter_context(tc.tile_pool(name=f"hp{it}", bufs=1))
                w1p = mctx.enter_context(tc.tile_pool(name=f"w1{it}", bufs=6))
                w2p = mctx.enter_context(tc.tile_pool(name=f"w2{it}", bufs=2))
                pp1 = mctx.enter_context(tc.tile_pool(
                    name=f"f1{it}", bufs=4, space="PSUM"))
                pp2 = mctx.enter_context(tc.tile_pool(
                    name=f"f2{it}", bufs=2, space="PSUM"))
                NF = 16
                # fb=0: FFN1 first half; FFN2 partials into y3t
                h_sb = [hp.tile([128, TQ], BF16, tag=f"h{i}",
                                name=f"h{i}") for i in range(NF)]
                for f in range(NF):
                    w1t = w1p.tile([128, NT, 128], BF16, tag="w1", name="w1")
                    nc.sync.dma_start(w1t[:], w_ap(t["wf1"], NT, f, 0, NT))
                    ps = pp1.tile([128, 512], F32, tag="p1", name="p1")
                    for ct in range(NT):
                        nc.tensor.matmul(ps[:], w1t[:, ct, :], qT2[ct][:],
                                         start=(ct == 0), stop=(ct == NT - 1))
                    fold_epilogue(ps, ffold, f, h_sb[f], AF.Relu, lv["bf1"])
                for co in range(NT):
                    w2t = w2p.tile([128, NF, 128], BF16, tag="w2", name="w2")
                    nc.sync.dma_start(w2t[:], w_ap(t["wf2"], FT, co, 0, NF))
                    ps = pp2.tile([128, 512], F32, tag="p2", name="p2")
                    for f in range(NF):
                        nc.tensor.matmul(ps[:], w2t[:, f, :], h_sb[f][:],
                                         start=(f == 0), stop=(f == NF - 1))
                    nc.vector.tensor_copy(y3t[:, co, :], ps[:])
                    norm2([co])  # LN2 normalize of y2 rides the fb0 window
                # fb=1: FFN1 second half; FFN2 in column chunks so each
                # chunk's LN3 chain + output DMA overlaps the next chunk's
                # matmuls on the PE.
                h_sb = [hp.tile([128, TQ], BF16, tag=f"h{i}",
                                name=f"h{i}") for i in range(NF)]
                w2h = []
                for f in range(NF):
                    fg = NF + f
                    w1t = w1p.tile([128, NT, 128], BF16, tag="w1", name="w1")
                    nc.sync.dma_start(w1t[:], w_ap(t["wf1"], NT, fg, 0, NT))
                    ps = pp1.tile([128, 512], F32, tag="p1", name="p1")
                    for ct in range(NT):
                        nc.tensor.matmul(ps[:], w1t[:, ct, :], qT2[ct][:],
                                         start=(ct == 0), stop=(ct == NT - 1))
                    fold_epilogue(ps, ffold, fg, h_sb[f], AF.Relu, lv["bf1"])
                    if f % 2 == 0:
                        co = f // 2
                        w2t = w2p.tile([128, NF, 128], BF16, tag=f"wc{co}",
                                       name=f"wc{co}", bufs=1)
                        nc.sync.dma_start(
                            w2t[:], w_ap(t["wf2"], FT, co, NF, NF))
                        w2h.append(w2t)
                for ca, cbnd in ((0, 288), (288, 512)):
                    sl = slice(ca, cbnd)
                    wch = cbnd - ca
                    for co in range(NT):
                        psf = pp2.tile([128, 512], F32, tag="p2", name="p2")
                        ps = psf[:, 0:wch]
                        for f in range(NF):
                            nc.tensor.matmul(ps[:], w2h[co][:, f, :],
                                             h_sb[f][:, sl],
                                             start=(f == 0),
                                             stop=(f == NF - 1))
                        nc.vector.scalar_tensor_tensor(
                            out=y3t[:, co, sl], in0=ps[:],
                            scalar=lv["bf2"][:, co:co + 1],
                            in1=y3t[:, co, sl],
                            op0=ALU.add, op1=ALU.add)
                        with nc.allow_low_precision(reason="bf16 residual"):
                            nc.vector.tensor_add(y2[co][:, sl],
                                                 y3t[:, co, sl],
                                                 y2[co][:, sl])
                        ln_feed(lnst3, y2[co], co, sl)
                    ln_chunk(lnst3, y2, lv["g3"], lv["b3"], y3t, sl)


def _shard(inputs):
    import ml_dtypes
    BF = ml_dtypes.bfloat16
    x = np.asarray(inputs["x"], dtype=np.float32)
    enc = np.asarray(inputs["enc_out"], dtype=np.float32)
    tpad = np.asarray(inputs["tgt_pad_mask"]).astype(np.float32)
    spad = np.asarray(inputs["src_pad_mask"]).astype(np.float32)
    ws = {k: np.asarray(inputs[k], dtype=np.float32)
          for k in ("Wq1", "Wk1", "Wv1", "Wo1", "Wq2", "Wk2", "Wv2", "Wo2",
                    "Wf1", "Wf2")}
    lnv = {k: np.asarray(inputs[k], dtype=np.float32)
           for k in ("ln1_g", "ln1_b", "ln2_g", "ln2_b", "ln3_g", "ln3_b",
                     "bf1", "bf2")}

    def pret(W):  # [cin, cout] -> [ot, p, ct, o] pretiled bf16
        cin, cout = W.shape
        return np.ascontiguousarray(
            W.reshape(cin // 128, 128, cout // 128, 128)
            .transpose(2, 1, 0, 3).astype(BF))

    # LN1 affine folded through Wq2; LN2 affine folded through Wf1.
    wq2f = lnv["ln1_g"][:, None] * ws["Wq2"]
    csq2 = wq2f.astype(BF).astype(np.float32).sum(axis=0)
    bq2 = lnv["ln1_b"] @ ws["Wq2"]
    wf1f = lnv["ln2_g"][:, None] * ws["Wf1"]
    csf1 = wf1f.astype(BF).astype(np.float32).sum(axis=0)
    bf1f = lnv["bf1"] + lnv["ln2_b"] @ ws["Wf1"]

    wt = {"wq1": pret(ws["Wq1"]), "wk1": pret(ws["Wk1"]),
          "wo1": pret(ws["Wo1"]), "wq2": pret(wq2f), "wk2": pret(ws["Wk2"]),
          "wo2": pret(ws["Wo2"]), "wf1": pret(wf1f), "wf2": pret(ws["Wf2"]),
          "wv1": np.ascontiguousarray(ws["Wv1"].astype(BF)),
          "wv2": np.ascontiguousarray(ws["Wv2"].astype(BF))}

    def cols(v):  # length n -> [128, n//128]
        return np.asarray(v, np.float32).reshape(-1, 128).T

    cblk = np.zeros((128, NCOL), np.float32)
    for k, vec in (("pad1", 1.0 - tpad[0]), ("pad2", 1.0 - spad[0]),
                   ("g1", lnv["ln1_g"]), ("b1", lnv["ln1_b"]),
                   ("g2", lnv["ln2_g"]), ("b2", lnv["ln2_b"]),
                   ("g3", lnv["ln3_g"]), ("b3", lnv["ln3_b"]),
                   ("bf1", bf1f), ("csq2", csq2), ("bq2", bq2),
                   ("csf1", csf1), ("bf2", lnv["bf2"])):
        c0, n = COLS[k]
        if k not in ("pad1", "pad2"):
            cblk[:, c0:c0 + n] = cols(vec)

    in_maps = []
    for b in range(B):
        xTb = np.ascontiguousarray(x[b].T.astype(BF))
        eTb = np.ascontiguousarray(enc[b].T.astype(BF))
        p1v, p2v = 1.0 - tpad[b], 1.0 - spad[b]
        for h in range(2):
            xTlb = np.ascontiguousarray(x[b, h::2, :].T.astype(BF))
            trih = np.ascontiguousarray(
                (np.arange(128)[:, None] <= 2 * np.arange(64)[None, :] + h
                 ).astype(BF))
            cb = cblk.copy()
            cb[:, 0:8] = cols(p1v)
            cb[:, 8:16] = cols(p2v)
            cb[:, 152:184] = trih.view(np.float32)
            m = {"xT": xTb, "xTl": xTlb, "encT": eTb,
                 "cblk": np.ascontiguousarray(cb)}
            m.update(wt)
            in_maps.append(m)
    return in_maps


def _get_nc(repeat=1):
    if repeat not in _CACHE:
        _CACHE[repeat] = _build(repeat)
    return _CACHE[repeat]


def kernel(**inputs):
    from concourse.bass_utils import run_bass_kernel_spmd
    nc = _get_nc()
    in_maps = _shard(inputs)
    res = run_bass_kernel_spmd(nc, in_maps, core_ids=list(range(8)))
    out = np.empty((B, T, C), np.float32)
    for core in range(8):
        b, h = core // 2, core % 2
        out[b, h::2, :] = res.results[core]["outT"].T
    return out


# revision 114
# speedup vs baseline: 1.0020x; 1.0020x over previous
"""Decoder block kernel for 8 Trainium2 NeuronCores.

Sharding: core = 2*b + h handles batch b, query tokens q with q % 2 == h
(interleaved so the causal-mask block structure is identical on every
core -> one SPMD program; the mask diagonal band differs only in DATA).

All activations live transposed [C, tokens] (C on partitions), so every
linear layer uses the stored [in,out] weights directly as the stationary
operand and no on-device transposes are needed. LayerNorm statistics are
computed with ones-matmuls on the PE (replicated across partitions);
softmax row sums come from a ones-column appended to V.

v2: matmul operands and streamed tensors are bfloat16 (weights pre-tiled
on the host so every DMA moves >=512B contiguous runs); LN/softmax stat
chains, folds and the final output stay fp32. Causal score/AV ranges are
trimmed to exact 64-column boundaries (no fp32r >=256-column constraint
with bf16 moving operands). Scalar constants ship as one packed block.
"""
import numpy as np

B, T, C, H, D, FF = 4, 1024, 1024, 16, 64, 4096
NT = C // 128   # 8 partition tiles of the model dim
KT = T // 128   # 8 context-token tiles
FT = FF // 128  # 32
TQ = T // 2     # 512 local query tokens per core

# packed fp32 const block: name -> (col0, ncols)
COLS = {"pad1": (0, 8), "pad2": (8, 8), "g1": (16, 8), "b1": (24, 8),
        "g2": (32, 8), "b2": (40, 8), "g3": (48, 8), "b3": (56, 8),
        "bf1": (64, 32), "csq2": (96, 8), "bq2": (104, 8),
        "csf1": (112, 32), "bf2": (144, 8), "tri": (152, 32)}
NCOL = 184

_CACHE = {}


def _build(repeat=1):
    import concourse.bacc as bacc
    import concourse.tile as tile
    from concourse import mybir

    # The act-table placement pass assigns each Activation the FIRST table
    # set containing its function. This kernel only uses Exp/Ln/Identity/
    # Relu/Copy, which coexist in the natural_log_exp_and_others set —
    # hiding those funcs from the OTHER sets (keeping list order, since
    # act_func_set_id is positional in act_info.json) makes the whole
    # kernel run off that one table: no 1.3us LoadActFuncSet reloads
    # between softmax Exp and the LN chains.
    _orig_tables = bacc.get_activation_tables

    def _tables_ln_exp_only(arch):
        tabs = _orig_tables(arch)
        key = "natural_log_exp_and_others"
        if key not in tabs:
            return tabs
        mine = {f for f in tabs[key]
                if f.name in ("Exp", "Ln", "Identity", "Relu", "Copy",
                              "Square")}
        return {k: (v if k == key else v - mine) for k, v in tabs.items()}

    bacc.get_activation_tables = _tables_ln_exp_only
    try:
        nc = _build_inner(bacc, tile, mybir, repeat)
    finally:
        bacc.get_activation_tables = _orig_tables
    return nc


def _build_inner(bacc, tile, mybir, repeat):
    nc = bacc.Bacc(None, target_bir_lowering=False)
    F32 = mybir.dt.float32
    BF16 = mybir.dt.bfloat16

    def din(name, shape, dt=BF16):
        return nc.dram_tensor(name, shape, dt, kind="ExternalInput")

    t = {}
    t["xT"] = din("xT", [C, T])
    t["xTl"] = din("xTl", [C, TQ])
    t["encT"] = din("encT", [C, T])
    # pre-tiled [ot, p, ct, o] layouts
    for k in ("wq1", "wk1", "wo1", "wq2", "wk2", "wo2"):
        t[k] = din(k, [C, C])
    t["wf1"] = din("wf1", [C, FF])        # pre-scaled by diag(g2), pretiled
    t["wf2"] = din("wf2", [FF, C])        # pretiled
    # natural [in, out] layouts (moving operand of the V matmul)
    t["wv1"] = din("wv1", [C, C])
    t["wv2"] = din("wv2", [C, C])
    t["cblk"] = din("cblk", [128, NCOL], F32)
    t["outT"] = nc.dram_tensor("outT", [C, TQ], F32, kind="ExternalOutput")

    with tile.TileContext(nc) as tc:
        for it in range(repeat):
            _emit(nc, tc, t, it)
    nc.compile()
    return nc



def _emit(nc, tc, t, it):
    from contextlib import ExitStack
    import concourse.bass as bass
    from concourse import mybir
    from concourse.tile import add_dep_helper

    F32 = mybir.dt.float32
    F32R = mybir.dt.float32r
    BF16 = mybir.dt.bfloat16
    AF = mybir.ActivationFunctionType
    ALU = mybir.AluOpType

    def w_ap(wdram, nctt, ot, a0, na):
        """pretiled arr[ot, p, ct, o]: [128, na, 128] view at (ot, a0)"""
        return bass.AP(tensor=wdram, offset=ot * nctt * 128 * 128 + a0 * 128,
                       ap=[[nctt * 128, 128], [128, na], [1, 128]])

    with ExitStack() as ctx:
        consts = ctx.enter_context(tc.tile_pool(name=f"con{it}", bufs=1))
        cb = consts.tile([128, NCOL], F32, tag="cb", name="cb")
        cb_dma = [None]

        def emit_cb(after=None):
            di = nc.scalar.dma_start(cb[:], t["cblk"][:])
            if after is not None:
                add_dep_helper(di.ins, after.ins,
                               reason="consts after critical startup stream")
            cb_dma[0] = di
        tri_sb = cb[:, COLS["tri"][0]:COLS["tri"][0] + 32].bitcast(BF16)
        ones128 = consts.tile([128, 128], BF16, tag="o128", name="o128")
        nc.vector.memset(ones128[:], 1.0)
        ones1 = consts.tile([1, 128], F32R, tag="o1", name="o1")
        nc.vector.memset(ones1[:].bitcast(F32), 1.0)
        eps_t = consts.tile([128, 1], F32, tag="eps", name="eps")
        nc.vector.memset(eps_t[:], 1e-5)

        lv = {k: cb[:, c0:c0 + n] for k, (c0, n) in COLS.items()}
        pad_sb = {"pad1": lv["pad1"], "pad2": lv["pad2"]}

        wpool = ctx.enter_context(tc.tile_pool(name=f"wp{it}", bufs=8))
        ypool = ctx.enter_context(tc.tile_pool(name=f"yp{it}", bufs=1))
        foldp = ctx.enter_context(tc.tile_pool(name=f"fp{it}", bufs=3))

        def fold_epilogue(ps, fold, ot, out_tile, func, bias_sb):
            """out = func(rstd*(ps - m*CS[ot]) + bias) given fold=(rstd, un)
            with un = -m*rstd, CS per-output-channel colsum."""
            rstd, un, cs = fold
            # PSUM reads and TensorScalarPtr are DVE/Act-only ops; GPSIMD
            # handles neither, so the whole fold chain stays on DVE.
            ftile = foldp.tile([128, TQ], F32, tag="ft", name="ft")
            nc.vector.tensor_mul(ftile[:], ps[:], rstd[:])
            nc.vector.scalar_tensor_tensor(
                out=ftile[:], in0=un[:], scalar=cs[:, ot:ot + 1], in1=ftile[:],
                op0=ALU.mult, op1=ALU.add)
            with nc.allow_low_precision(reason="bf16 activations"):
                nc.scalar.activation(out_tile[:], ftile[:], func,
                                     bias=bias_sb[:, ot:ot + 1])

        def new_y(dt=BF16):
            return [ypool.tile([128, TQ], dt, tag=f"y{i}", name=f"y{i}")
                    for i in range(NT)]

        def linear_T(wdram, cin, cout, in_tiles, n, epilogue, pp,
                     dma_out=None, first_cb=None):
            """psum[ot][:, q0:] = sum_ct W[ct,ot].T @ in[ct][:, q0:]"""
            nct = cin // 128
            for ot in range(cout // 128):
                wt = wpool.tile([128, nct, 128], BF16, tag="w", name="w")
                di = nc.sync.dma_start(wt[:], w_ap(wdram, nct, ot, 0, nct))
                if dma_out is not None:
                    dma_out.append(di)
                if ot == 0 and first_cb is not None:
                    first_cb()
                for q0 in range(0, n, 512):
                    ps = pp.tile([128, 512], F32, tag="pp", name="pp")
                    for ct in range(nct):
                        nc.tensor.matmul(ps[:], wt[:, ct, :],
                                         in_tiles[ct][:, q0:q0 + 512],
                                         start=(ct == 0), stop=(ct == nct - 1))
                    epilogue(ot, q0, ps)

        def ln_begin(sctx, tagp, lnps, lntag):
            sqp = sctx.enter_context(tc.tile_pool(name=f"sq{tagp}{it}", bufs=2))
            scr = sctx.enter_context(tc.tile_pool(name=f"ls{tagp}{it}", bufs=1))
            ps1 = lnps.tile([128, TQ], F32, tag=lntag, name="ln")
            ps2 = lnps.tile([128, TQ], F32, tag=lntag, name="ln")
            return {"sqp": sqp, "scr": scr, "ps1": ps1, "ps2": ps2}

        def ln_feed(st, y_tile, ct, sl=slice(0, TQ), sq_eng=None):
            """Accumulate sum(y) and sum(y^2) for one partition tile."""
            w = sl.stop - sl.start
            sq = st["sqp"].tile([128, TQ], BF16, tag="sq", name="sq")
            with nc.allow_low_precision(reason="bf16 squares ok"):
                (sq_eng or nc.vector).tensor_mul(sq[:, 0:w], y_tile[:, sl],
                                                 y_tile[:, sl])
            nc.tensor.matmul(st["ps1"][:, sl], ones128[:], y_tile[:, sl],
                             start=(ct == 0), stop=(ct == NT - 1),
                             skip_group_check=True)
            nc.tensor.matmul(st["ps2"][:, sl], ones128[:], sq[:, 0:w],
                             start=(ct == 0), stop=(ct == NT - 1),
                             skip_group_check=True)

        def ln_finish(st, y_in, g, b, out_t, statpool):
            """Stats chain now; the per-tile normalize (overwrites y_in) is
            returned as a closure so the caller can emit it where the vector
            engines are idle. Returns ((rstd, un), emit_norm)."""
            sp_ = statpool
            m = sp_.tile([128, TQ], F32, tag="m", name="m")
            nc.scalar.mul(m[:], st["ps1"][:], 1.0 / C)
            m2 = sp_.tile([128, TQ], F32, tag="v2", name="v2")
            nc.scalar.activation(m2[:], m[:], AF.Square)
            ms = sp_.tile([128, TQ], F32, tag="v", name="v")
            nc.vector.tensor_scalar_mul(ms[:], st["ps2"][:], 1.0 / C)
            nc.vector.tensor_sub(ms[:], ms[:], m2[:])
            # rstd = (v+eps)^-0.5 = exp(-0.5*ln(v+eps)): Ln+Exp live in the
            # same activation-table set as attention's Exp, so the Act engine
            # never reloads tables (Sqrt would force a 1.3us switch).
            nc.scalar.activation(ms[:], ms[:], AF.Ln, bias=eps_t[:])
            rstd = sp_.tile([128, TQ], F32, tag="r", name="r")
            nc.scalar.activation(rstd[:], ms[:], AF.Exp, scale=-0.5)
            un = sp_.tile([128, TQ], F32, tag="un", name="un")
            nc.vector.scalar_tensor_tensor(
                out=un[:], in0=m[:], scalar=-1.0, in1=rstd[:],
                op0=ALU.mult, op1=ALU.mult)

            def emit_norm(cts):
                for ct in cts:
                    eng = nc.gpsimd if ct in (3, 7) else nc.vector
                    d = foldp.tile([128, TQ], F32, tag="dn", name="dn",
                                   bufs=4)
                    eng.tensor_sub(d[:], y_in[ct][:], m[:])
                    eng.tensor_mul(d[:], d[:], rstd[:])
                    with nc.allow_low_precision(reason="bf16 activations"):
                        nc.scalar.activation(out_t[ct][:], d[:], AF.Identity,
                                             bias=b[:, ct:ct + 1],
                                             scale=g[:, ct:ct + 1])
            return (rstd, un), emit_norm

        def ln_chunk(st, y_in, g, b, y3t, sl):
            """Finish one column chunk: stats chain + normalize + out DMA."""
            scr = st["scr"]
            w = sl.stop - sl.start
            if "m" not in st:
                st["m"] = scr.tile([128, TQ], F32, tag="m", name="m", bufs=1)
                st["v"] = scr.tile([128, TQ], F32, tag="v", name="v", bufs=1)
                st["v2"] = scr.tile([128, TQ], F32, tag="v2", name="v2",
                                    bufs=1)
                st["r"] = scr.tile([128, TQ], F32, tag="r", name="r", bufs=1)
            m, ms, m2, rstd = st["m"], st["v"], st["v2"], st["r"]
            # m via Act (PSUM-read ok) in parallel with ms on DVE, then the
            # square also on Act back-to-back — one less cross-engine hop
            nc.scalar.mul(m[:, sl], st["ps1"][:, sl], 1.0 / C)
            nc.scalar.activation(m2[:, sl], m[:, sl], AF.Square)
            nc.vector.tensor_scalar_mul(ms[:, sl], st["ps2"][:, sl], 1.0 / C)
            nc.vector.tensor_sub(ms[:, sl], ms[:, sl], m2[:, sl])
            nc.scalar.activation(ms[:, sl], ms[:, sl], AF.Ln, bias=eps_t[:])
            nc.scalar.activation(rstd[:, sl], ms[:, sl], AF.Exp, scale=-0.5)
            last = True
            for ct in range(NT):
                eng = nc.vector if ct % 2 == 0 else nc.gpsimd
                d = scr.tile([128, w], F32, tag="dc", name="dc", bufs=6)
                eng.tensor_sub(d[:], y_in[ct][:, sl], m[:, sl])
                eng.tensor_mul(d[:], d[:], rstd[:, sl])
                nc.scalar.activation(y3t[:, ct, sl], d[:], AF.Identity,
                                     bias=b[:, ct:ct + 1],
                                     scale=g[:, ct:ct + 1])
                if last and ct == 3:
                    nc.sync.dma_start(
                        bass.AP(tensor=t["outT"], offset=sl.start,
                                ap=[[TQ, 128], [128 * TQ, 4], [1, w]]),
                        y3t[:, 0:4, sl])
            if last:
                nc.sync.dma_start(
                    bass.AP(tensor=t["outT"], offset=4 * 128 * TQ + sl.start,
                            ap=[[TQ, 128], [128 * TQ, 4], [1, w]]),
                    y3t[:, 4:8, sl])
            else:
                nc.sync.dma_start(
                    bass.AP(tensor=t["outT"], offset=sl.start,
                            ap=[[TQ, 128], [128 * TQ, NT], [1, w]]),
                    y3t[:, :, sl])

        def kv_stage(src, wk_d, wv_d, wq_d, q_src, padkey, kTt, vtt,
                     qTt, tagp, q_first, pp, qfold=None, src_dma=None,
                     after_k=None, mid_hook=None, q_first_cb=None):
            """Compute K_T, V (pad-masked, with the pad column in slot 64 for
            the softmax row sums), and Q_T from a resident transposed source.
            q_first=True: emit Q's matmuls before K (source already resident,
            weight DMAs returned through src_dma ordering); False: K first so
            the PE can run while the previous LN chain finishes."""
            with ExitStack() as kctx:
                def emit_q(dma_out=None):
                    if qfold is None:
                        def ep(ot, q0, ps):
                            with nc.allow_low_precision(reason="bf16 acts"):
                                nc.scalar.copy(qTt[ot][:], ps[:])
                        linear_T(wq_d, C, C, q_src, TQ, ep, pp,
                                 dma_out=dma_out, first_cb=q_first_cb)
                    else:
                        linear_T(wq_d, C, C, q_src, TQ,
                                 lambda ot, q0, ps: fold_epilogue(
                                     ps, qfold, ot, qTt[ot], AF.Identity,
                                     lv["bq2"]), pp, dma_out=dma_out)

                if q_first:
                    wd = []
                    emit_q(dma_out=wd)
                    if src_dma is not None:
                        src_dma(wd)

                def kep(ot, q0, ps):
                    # PSUM source: only DVE/Act may read PSUM (not GPSIMD)
                    with nc.allow_low_precision(reason="bf16 acts"):
                        nc.vector.tensor_copy(kTt[ot][:, q0:q0 + 512], ps[:])
                kd = []
                linear_T(wk_d, C, C, src, T, kep, pp, dma_out=kd)
                if after_k is not None:
                    after_k(kd)
                if not q_first:
                    emit_q()
                if mid_hook is not None:
                    mid_hook()
                wvp = kctx.enter_context(tc.tile_pool(name=f"wv{tagp}{it}",
                                                      bufs=2))
                for tt in range(KT):
                    with nc.allow_low_precision(reason="bf16 pad col"):
                        nc.gpsimd.tensor_scalar_mul(
                            vtt[tt][:, :, 64:65], ones128[:, 0:16],
                            pad_sb[padkey][:, tt:tt + 1])
                for half in range(2):
                    wvq = []
                    for cq in range(4):
                        wvt = wvp.tile([128, 2, 512], BF16, tag="wv",
                                       name="wv", bufs=8)
                        nc.sync.dma_start(
                            wvt[:],
                            bass.AP(tensor=wv_d,
                                    offset=512 * half + 256 * cq * C,
                                    ap=[[C, 128], [128 * C, 2], [1, 512]]))
                        wvq.append(wvt)
                    for tt in range(KT):
                        ps = pp.tile([128, 512], F32, tag="pp", name="pp")
                        for ct in range(NT):
                            nc.tensor.matmul(
                                ps[:], src[ct][:, 128 * tt:128 * (tt + 1)],
                                wvq[ct // 2][:, ct % 2, :],
                                start=(ct == 0), stop=(ct == NT - 1))
                        with nc.allow_low_precision(reason="bf16 acts"):
                            nc.vector.tensor_scalar_mul(
                                vtt[tt][:, 8 * half:8 * (half + 1), 0:64],
                                ps[:].rearrange("p (h d) -> p h d", d=64),
                                pad_sb[padkey][:, tt:tt + 1])

        def attention(qTt, kTt, vtt, wo_d, resid, g, b, out_t, causal,
                      sctx, tagp, scp, avp, rbp, op, statpool,
                      paired=False):
            """Scores/AV in transposed layout; O-projection output (+resid)
            is written back into the qT tiles (dead by then), then LN."""
            with ExitStack() as atx:
                pvp = atx.enter_context(tc.tile_pool(
                    name=f"pv{tagp}{it}", bufs=1))
                ppool = atx.enter_context(tc.tile_pool(
                    name=f"pt{tagp}{it}", bufs=8))
                sbp = atx.enter_context(tc.tile_pool(
                    name=f"sb{tagp}{it}", bufs=2))
                pv = [pvp.tile([128, TQ], BF16, tag=f"pv{i}", name=f"pv{i}")
                      for i in range(NT)]
                hseq = [(h0 + 4) % H for h0 in range(H)]
                for hpos, h in enumerate(hseq):
                    ct, off = h // 2, (h % 2) * 64
                    av = avp.tile([65, 512], F32, tag="av", name="av")
                    pend = None

                    def emit_av(p, c, k, av=av, h=h):
                        nc.tensor.matmul(av[:, c:512], vtt[k][:, h, 0:65],
                                         p[:, c:512],
                                         start=(k == 0), stop=(k == KT - 1))

                    if paired:
                        # score two key tiles into one 2-bank PSUM tile and
                        # exponentiate both with ONE Act instruction (Act
                        # per-instr overhead pins the attention windows).
                        # Causal pairs (k+4, k): the longer-range tile sits
                        # first so one exp range [64(k+4), 1024) covers both
                        # valid regions (the unwritten [512, 512+64k) sliver
                        # is exp'd but never read). AV for kt=0 is emitted
                        # first within its pair: its full-span matmul carries
                        # the PSUM-initializing start flag.
                        pairs = ([(k + 4, k) for k in range(4)] if causal
                                 else [(2 * pk, 2 * pk + 1)
                                       for pk in range(4)])

                        def emit_pair_avs(Pt2, ka, kb):
                            order = ((1, kb), (0, ka)) if causal                                 else ((0, ka), (1, kb))
                            for j, kt in order:
                                emit_av(Pt2[:, 512 * j:512 * (j + 1)],
                                        64 * kt if causal else 0, kt)

                        for ka, kb in pairs:
                            sp2 = scp.tile([128, 1024], F32, tag="sc",
                                           name="sc")
                            Pt2 = ppool.tile([128, 1024], BF16, tag="P",
                                             name="P")
                            for j, kt in ((0, ka), (1, kb)):
                                c0 = 64 * kt if causal else 0
                                nc.tensor.matmul(
                                    sp2[:, 512 * j + c0:512 * (j + 1)],
                                    kTt[ct][off:off + 64,
                                            128 * kt:128 * (kt + 1)],
                                    qTt[ct][off:off + 64, c0:512],
                                    start=True, stop=True)
                            e0 = 64 * ka if causal else 0
                            with nc.allow_low_precision(reason="bf16 probs"):
                                nc.scalar.activation(Pt2[:, e0:1024],
                                                     sp2[:, e0:1024],
                                                     AF.Exp, scale=0.125)
                            if causal:
                                for j, kt in ((0, ka), (1, kb)):
                                    d0 = 512 * j + 64 * kt
                                    nc.gpsimd.tensor_mul(
                                        Pt2[:, d0:d0 + 64],
                                        Pt2[:, d0:d0 + 64], tri_sb[:])
                            if pend is not None:
                                emit_pair_avs(*pend)
                            pend = (Pt2, ka, kb)
                        emit_pair_avs(*pend)
                    else:
                        for kt in range(KT):
                            c0 = 64 * kt if causal else 0
                            sp = scp.tile([128, 512], F32, tag="pp",
                                          name="pp")
                            Pt = ppool.tile([128, TQ], BF16, tag="P",
                                            name="P")
                            nc.tensor.matmul(
                                sp[:, c0:512],
                                kTt[ct][off:off + 64,
                                        128 * kt:128 * (kt + 1)],
                                qTt[ct][off:off + 64, c0:512],
                                start=True, stop=True)
                            with nc.allow_low_precision(reason="bf16 probs"):
                                nc.scalar.activation(Pt[:, c0:512],
                                                     sp[:, c0:512],
                                                     AF.Exp, scale=0.125)
                            if causal:
                                nc.gpsimd.tensor_mul(
                                    Pt[:, 64 * kt:64 * (kt + 1)],
                                    Pt[:, 64 * kt:64 * (kt + 1)], tri_sb[:])
                            if pend is not None:
                                emit_av(*pend)
                            pend = (Pt, c0, kt)
                        emit_av(*pend)
                    rinv = sbp.tile([1, 512], F32R, tag="ri", name="ri")
                    with nc.allow_low_precision(reason="fp32r rounding ok"):
                        nc.vector.reciprocal(rinv[:], av[64:65, :])
                    if rbp is not None:
                        rb_ps = rbp.tile([128, 512], F32, tag="rb",
                                         name="rb")
                    else:
                        rb_ps = op.tile([128, 512], F32, tag="o", name="o")
                    nc.tensor.matmul(rb_ps[:], ones1[:], rinv[:],
                                     start=True, stop=True)
                    if hpos >= H - 2:
                        # tail heads: lift AV out of PSUM on the (now idle)
                        # Act engine in parallel with the reciprocal, and
                        # multiply against the PSUM broadcast directly — one
                        # DVE hop shorter, so WO isn't held up.
                        avs = sbp.tile([64, 512], F32, tag="avs", name="avs")
                        nc.scalar.copy(avs[:], av[0:64, :])
                        with nc.allow_low_precision(reason="bf16 acts"):
                            nc.vector.tensor_mul(pv[ct][off:off + 64, :],
                                                 avs[:], rb_ps[0:64, :])
                    else:
                        rb = sbp.tile([64, 512], F32, tag="rs", name="rs")
                        nc.vector.tensor_copy(rb[:], rb_ps[0:64, :])
                        with nc.allow_low_precision(reason="bf16 acts"):
                            nc.vector.tensor_mul(pv[ct][off:off + 64, :],
                                                 av[0:64, :], rb[:])
                lnst = ln_begin(atx, tagp, avp, "av")
                for co in range(NT):
                    wt = wpool.tile([128, NT, 128], BF16, tag="w", name="w")
                    nc.sync.dma_start(wt[:], w_ap(wo_d, NT, co, 0, NT))
                    ps = op.tile([128, 512], F32, tag="o", name="o")
                    for i, cc in enumerate((c + 2) % NT for c in range(NT)):
                        nc.tensor.matmul(ps[:], wt[:, cc, :], pv[cc][:],
                                         start=(i == 0), stop=(i == NT - 1))
                    with nc.allow_low_precision(reason="bf16 acts"):
                        nc.vector.tensor_add(qTt[co][:], ps[:], resid[co][:])
                    ln_feed(lnst, qTt[co], co)
                return ln_finish(lnst, qTt, g, b, out_t, statpool)

        # ================= main flow =================
        qpool = ctx.enter_context(tc.tile_pool(name=f"qp{it}", bufs=1))
        statp = ctx.enter_context(tc.tile_pool(name=f"st{it}", bufs=1))

        def new_q(pfx="q"):
            return [qpool.tile([128, TQ], BF16, tag=f"{pfx}{i}",
                               name=f"{pfx}{i}") for i in range(NT)]

        with ExitStack() as actx:
            qkv = actx.enter_context(tc.tile_pool(name=f"qkv{it}", bufs=1))
            srcp = actx.enter_context(tc.tile_pool(name=f"sr{it}", bufs=1))
            p1 = actx.enter_context(ExitStack())
            pps = p1.enter_context(tc.tile_pool(
                name=f"ps{it}", bufs=3, space="PSUM"))
            scs = p1.enter_context(tc.tile_pool(
                name=f"ss{it}", bufs=3, space="PSUM"))
            avps = p1.enter_context(tc.tile_pool(
                name=f"as{it}", bufs=2, space="PSUM"))

            def new_kv():
                k = [qkv.tile([128, T], BF16, tag=f"k{i}", name=f"k{i}")
                     for i in range(NT)]
                v = [qkv.tile([128, 16, 65], BF16, tag=f"v{i}", name=f"v{i}")
                     for i in range(KT)]
                return k, v

            def load_src(dram, tag, eng, after=None):
                st_ = srcp.tile([128, NT, T], BF16, tag=tag, name=tag)
                dmas = []
                for th in range(2):
                    di = eng.dma_start(
                        st_[:, :, 512 * th:512 * (th + 1)],
                        bass.AP(tensor=dram, offset=512 * th,
                                ap=[[T, 128], [128 * T, NT], [1, 512]]))
                    if after is not None and th < len(after):
                        add_dep_helper(di.ins, after[th].ins,
                                       reason="src after critical weights")
                    dmas.append(di)
                return [st_[:, i, :] for i in range(NT)], dmas

            # ---- self-attention + AddNorm ----
            qT = new_q()
            kTt, vtt = new_kv()
            with ExitStack() as sctx:
                xt_t = qpool.tile([128, NT, TQ], BF16, tag="xt", name="xt")
                xTl_sb = [xt_t[:, i, :] for i in range(NT)]

                def xt_emit():
                    # emitted after the first wq1 tile DMA so the weight
                    # stream heads the startup DMA queue; first ct alone so
                    # the very first matmul can fire early
                    dis = []
                    for c0_, nc_ in ((0, 1), (1, 3), (4, 4)):
                        dis.append(nc.sync.dma_start(
                            xt_t[:, c0_:c0_ + nc_, :],
                            bass.AP(tensor=t["xTl"], offset=c0_ * 128 * TQ,
                                    ap=[[TQ, 128], [128 * TQ, nc_], [1, TQ]])))
                    emit_cb(after=dis[0])
                src1, src2 = [None], [None]

                def src_dma(wd):
                    src1[0] = load_src(t["xT"], "sA", nc.gpsimd,
                                       after=[wd[1], wd[3]])[0]

                def enc_dma(kd):
                    # prefetch encT on the gpsimd queue once the K-weight
                    # stream (critical for the current stage) is in flight
                    src2[0] = load_src(t["encT"], "sB", nc.gpsimd,
                                       after=[kd[3], kd[7]])[0]

                class _SrcProxy:
                    def __getitem__(self, i):
                        return src1[0][i]
                kv_stage(_SrcProxy(), t["wk1"], t["wv1"], t["wq1"], xTl_sb,
                         "pad1", kTt, vtt, qT, "s", True, pps,
                         src_dma=src_dma, after_k=enc_dma,
                         q_first_cb=xt_emit)
                y1 = new_y()
                fold1, norm1 = attention(qT, kTt, vtt, t["wo1"], xTl_sb,
                                         lv["g1"], lv["b1"], y1, True,
                                         sctx, "s", bigp, avps, None, ops,
                                         statpool=statp, paired=True)

            # ---- cross-attention + AddNorm (fresh tile generations) ----
            qT2 = new_q("x")
            kTt2, vtt2 = new_kv()
            with ExitStack() as cctx:
                kv_stage(src2[0], t["wk2"], t["wv2"], t["wq2"], qT,
                         "pad2", kTt2, vtt2, qT2, "c", False, pps,
                         qfold=(fold1[0], fold1[1], lv["csq2"]),
                         mid_hook=lambda: norm1(range(NT)))
                # release the self-stage PSUM pools so cross-attention can
                # afford 2-bank paired score tiles (no PE filler competes
                # for PSUM in this window)
                p1.close()
                sc2ps = cctx.enter_context(tc.tile_pool(
                    name=f"sc{it}", bufs=2, space="PSUM"))
                avps2 = cctx.enter_context(tc.tile_pool(
                    name=f"a2{it}", bufs=2, space="PSUM"))
                ops2 = cctx.enter_context(tc.tile_pool(
                    name=f"o2{it}", bufs=2, space="PSUM"))
                y2 = new_y()
                fold2, norm2 = attention(qT2, kTt2, vtt2, t["wo2"], y1,
                                         lv["g2"], lv["b2"], y2, False,
                                         cctx, "c", sc2ps, avps2, None,
                                         ops2, statpool=statp, paired=True)

        # ---- FFN + AddNorm ----
        with ExitStack() as fctx:
            y3p = fctx.enter_context(tc.tile_pool(name=f"y3{it}", bufs=1))
            y3t = y3p.tile([128, NT, TQ], F32, tag="z", name="z")
            lnps3 = fctx.enter_context(tc.tile_pool(
                name=f"l3{it}", bufs=2, space="PSUM"))
            lnst3 = ln_begin(fctx, "f", lnps3, "ln")
            ffold = (fold2[0], fold2[1], lv["csf1"])
            with ExitStack() as mctx:
                hp = mctx.enter_context(tc.tile_pool(name=f"hp{it}", bufs=1))
                w1p = mctx.enter_context(tc.tile_pool(name=f"w1{it}", bufs=6))
                w2p = mctx.enter_context(tc.tile_pool(name=f"w2{it}", bufs=2))
                pp1 = mctx.enter_context(tc.tile_pool(
                    name=f"f1{it}", bufs=3, space="PSUM"))
                pp2 = mctx.enter_context(tc.tile_pool(
                    name=f"f2{it}", bufs=3, space="PSUM"))
                NF = 16
                # fb=0: FFN1 first half; FFN2 partials into y3t
                h_sb = [hp.tile([128, TQ], BF16, tag=f"h{i}",
                                name=f"h{i}") for i in range(NF)]
                for f in range(NF):
                    w1t = w1p.tile([128, NT, 128], BF16, tag="w1", name="w1")
                    nc.sync.dma_start(w1t[:], w_ap(t["wf1"], NT, f, 0, NT))
                    ps = pp1.tile([128, 512], F32, tag="p1", name="p1")
                    for ct in range(NT):
                        nc.tensor.matmul(ps[:], w1t[:, ct, :], qT2[ct][:],
                                         start=(ct == 0), stop=(ct == NT - 1))
                    fold_epilogue(ps, ffold, f, h_sb[f], AF.Relu, lv["bf1"])
                for co in range(NT):
                    w2t = w2p.tile([128, NF, 128], BF16, tag="w2", name="w2")
                    nc.sync.dma_start(w2t[:], w_ap(t["wf2"], FT, co, 0, NF))
                    ps = pp2.tile([128, 512], F32, tag="p2", name="p2")
                    for f in range(NF):
                        nc.tensor.matmul(ps[:], w2t[:, f, :], h_sb[f][:],
                                         start=(f == 0), stop=(f == NF - 1))
                    nc.vector.tensor_copy(y3t[:, co, :], ps[:])
                    norm2([co])  # LN2 normalize of y2 rides the fb0 window
                # fb=1: FFN1 second half; FFN2 in column chunks so each
                # chunk's LN3 chain + output DMA overlaps the next chunk's
                # matmuls on the PE.
                h_sb = [hp.tile([128, TQ], BF16, tag=f"h{i}",
                                name=f"h{i}") for i in range(NF)]
                w2h = []
                for f in range(NF):
                    fg = NF + f
                    w1t = w1p.tile([128, NT, 128], BF16, tag="w1", name="w1")
                    nc.sync.dma_start(w1t[:], w_ap(t["wf1"], NT, fg, 0, NT))
                    ps = pp1.tile([128, 512], F32, tag="p1", name="p1")
                    for ct in range(NT):
                        nc.tensor.matmul(ps[:], w1t[:, ct, :], qT2[ct][:],
                                         start=(ct == 0), stop=(ct == NT - 1))
                    fold_epilogue(ps, ffold, fg, h_sb[f], AF.Relu, lv["bf1"])
                    if f % 2 == 0:
                        co = f // 2
                        w2t = w2p.tile([128, NF, 128], BF16, tag=f"wc{co}",
                                       name=f"wc{co}", bufs=1)
                        nc.sync.dma_start(
                            w2t[:], w_ap(t["wf2"], FT, co, NF, NF))
                        w2h.append(w2t)
                for ca, cbnd in ((0, 288), (288, 512)):
                    sl = slice(ca, cbnd)
                    wch = cbnd - ca
                    for co in range(NT):
                        psf = pp2.tile([128, 512], F32, tag="p2", name="p2")
                        ps = psf[:, 0:wch]
                        for f in range(NF):
                            nc.tensor.matmul(ps[:], w2h[co][:, f, :],
                                             h_sb[f][:, sl],
                                             start=(f == 0),
                                             stop=(f == NF - 1))
                        nc.vector.scalar_tensor_tensor(
                            out=y3t[:, co, sl], in0=ps[:],
                            scalar=lv["bf2"][:, co:co + 1],
                            in1=y3t[:, co, sl],
                            op0=ALU.add, op1=ALU.add)
                        with nc.allow_low_precision(reason="bf16 residual"):
                            nc.vector.tensor_add(y2[co][:, sl],
                                                 y3t[:, co, sl],
                                                 y2[co][:, sl])
                        ln_feed(lnst3, y2[co], co, sl)
                    ln_chunk(lnst3, y2, lv["g3"], lv["b3"], y3t, sl)


def _shard(inputs):
    import ml_dtypes
    BF = ml_dtypes.bfloat16
    x = np.asarray(inputs["x"], dtype=np.float32)
    enc = np.asarray(inputs["enc_out"], dtype=np.float32)
    tpad = np.asarray(inputs["tgt_pad_mask"]).astype(np.float32)
    spad = np.asarray(inputs["src_pad_mask"]).astype(np.float32)
    ws = {k: np.asarray(inputs[k], dtype=np.float32)
          for k in ("Wq1", "Wk1", "Wv1", "Wo1", "Wq2", "Wk2", "Wv2", "Wo2",
                    "Wf1", "Wf2")}
    lnv = {k: np.asarray(inputs[k], dtype=np.float32)
           for k in ("ln1_g", "ln1_b", "ln2_g", "ln2_b", "ln3_g", "ln3_b",
                     "bf1", "bf2")}

    def pret(W):  # [cin, cout] -> [ot, p, ct, o] pretiled bf16
        cin, cout = W.shape
        return np.ascontiguousarray(
            W.reshape(cin // 128, 128, cout // 128, 128)
            .transpose(2, 1, 0, 3).astype(BF))

    # LN1 affine folded through Wq2; LN2 affine folded through Wf1.
    wq2f = lnv["ln1_g"][:, None] * ws["Wq2"]
    csq2 = wq2f.astype(BF).astype(np.float32).sum(axis=0)
    bq2 = lnv["ln1_b"] @ ws["Wq2"]
    wf1f = lnv["ln2_g"][:, None] * ws["Wf1"]
    csf1 = wf1f.astype(BF).astype(np.float32).sum(axis=0)
    bf1f = lnv["bf1"] + lnv["ln2_b"] @ ws["Wf1"]

    wt = {"wq1": pret(ws["Wq1"]), "wk1": pret(ws["Wk1"]),
          "wo1": pret(ws["Wo1"]), "wq2": pret(wq2f), "wk2": pret(ws["Wk2"]),
          "wo2": pret(ws["Wo2"]), "wf1": pret(wf1f), "wf2": pret(ws["Wf2"]),
          "wv1": np.ascontiguousarray(ws["Wv1"].astype(BF)),
          "wv2": np.ascontiguousarray(ws["Wv2"].astype(BF))}

    def cols(v):  # length n -> [128, n//128]
        return np.asarray(v, np.float32).reshape(-1, 128).T

    cblk = np.zeros((128, NCOL), np.float32)
    for k, vec in (("pad1", 1.0 - tpad[0]), ("pad2", 1.0 - spad[0]),
                   ("g1", lnv["ln1_g"]), ("b1", lnv["ln1_b"]),
                   ("g2", lnv["ln2_g"]), ("b2", lnv["ln2_b"]),
                   ("g3", lnv["ln3_g"]), ("b3", lnv["ln3_b"]),
                   ("bf1", bf1f), ("csq2", csq2), ("bq2", bq2),
                   ("csf1", csf1), ("bf2", lnv["bf2"])):
        c0, n = COLS[k]
        if k not in ("pad1", "pad2"):
            cblk[:, c0:c0 + n] = cols(vec)

    in_maps = []
    for b in range(B):
        xTb = np.ascontiguousarray(x[b].T.astype(BF))
        eTb = np.ascontiguousarray(enc[b].T.astype(BF))
        p1v, p2v = 1.0 - tpad[b], 1.0 - spad[b]
        for h in range(2):
            xTlb = np.ascontiguousarray(x[b, h::2, :].T.astype(BF))
            trih = np.ascontiguousarray(
                (np.arange(128)[:, None] <= 2 * np.arange(64)[None, :] + h
                 ).astype(BF))
            cb = cblk.copy()
            cb[:, 0:8] = cols(p1v)
            cb[:, 8:16] = cols(p2v)
            cb[:, 152:184] = trih.view(np.float32)
            m = {"xT": xTb, "xTl": xTlb, "encT": eTb,
                 "cblk": np.ascontiguousarray(cb)}
            m.update(wt)
            in_maps.append(m)
    return in_maps


def _get_nc(repeat=1):
    if repeat not in _CACHE:
        _CACHE[repeat] = _build(repeat)
    return _CACHE[repeat]


def kernel(**inputs):
    from concourse.bass_utils import run_bass_kernel_spmd
    nc = _get_nc()
    in_maps = _shard(inputs)
    res = run_bass_kernel_spmd(nc, in_maps, core_ids=list(range(8)))
    out = np.empty((B, T, C), np.float32)
    for core in range(8):
        b, h = core // 2, core % 2
        out[b, h::2, :] = res.results[core]["outT"].T
    return out


# revision 115
# speedup vs baseline: 1.0030x; 1.0010x over previous
"""Decoder block kernel for 8 Trainium2 NeuronCores.

Sharding: core = 2*b + h handles batch b, query tokens q with q % 2 == h
(interleaved so the causal-mask block structure is identical on every
core -> one SPMD program; the mask diagonal band differs only in DATA).

All activations live transposed [C, tokens] (C on partitions), so every
linear layer uses the stored [in,out] weights directly as the stationary
operand and no on-device transposes are needed. LayerNorm statistics are
computed with ones-matmuls on the PE (replicated across partitions);
softmax row sums come from a ones-column appended to V.

v2: matmul operands and streamed tensors are bfloat16 (weights pre-tiled
on the host so every DMA moves >=512B contiguous runs); LN/softmax stat
chains, folds and the final output stay fp32. Causal score/AV ranges are
trimmed to exact 64-column boundaries (no fp32r >=256-column constraint
with bf16 moving operands). Scalar constants ship as one packed block.
"""
import numpy as np

B, T, C, H, D, FF = 4, 1024, 1024, 16, 64, 4096
NT = C // 128   # 8 partition tiles of the model dim
KT = T // 128   # 8 context-token tiles
FT = FF // 128  # 32
TQ = T // 2     # 512 local query tokens per core

# packed fp32 const block: name -> (col0, ncols)
COLS = {"pad1": (0, 8), "pad2": (8, 8), "g1": (16, 8), "b1": (24, 8),
        "g2": (32, 8), "b2": (40, 8), "g3": (48, 8), "b3": (56, 8),
        "bf1": (64, 32), "csq2": (96, 8), "bq2": (104, 8),
        "csf1": (112, 32), "bf2": (144, 8), "tri": (152, 32)}
NCOL = 184

_CACHE = {}


def _build(repeat=1):
    import concourse.bacc as bacc
    import concourse.tile as tile
    from concourse import mybir

    # The act-table placement pass assigns each Activation the FIRST table
    # set containing its function. This kernel only uses Exp/Ln/Identity/
    # Relu/Copy, which coexist in the natural_log_exp_and_others set —
    # hiding those funcs from the OTHER sets (keeping list order, since
    # act_func_set_id is positional in act_info.json) makes the whole
    # kernel run off that one table: no 1.3us LoadActFuncSet reloads
    # between softmax Exp and the LN chains.
    _orig_tables = bacc.get_activation_tables

    def _tables_ln_exp_only(arch):
        tabs = _orig_tables(arch)
        key = "natural_log_exp_and_others"
        if key not in tabs:
            return tabs
        mine = {f for f in tabs[key]
                if f.name in ("Exp", "Ln", "Identity", "Relu", "Copy",
                              "Square")}
        return {k: (v if k == key else v - mine) for k, v in tabs.items()}

    bacc.get_activation_tables = _tables_ln_exp_only
    try:
        nc = _build_inner(bacc, tile, mybir, repeat)
    finally:
        bacc.get_activation_tables = _orig_tables
    return nc


def _build_inner(bacc, tile, mybir, repeat):
    nc = bacc.Bacc(None, target_bir_lowering=False)
    F32 = mybir.dt.float32
    BF16 = mybir.dt.bfloat16

    def din(name, shape, dt=BF16):
        return nc.dram_tensor(name, shape, dt, kind="ExternalInput")

    t = {}
    t["xT"] = din("xT", [C, T])
    t["xTl"] = din("xTl", [C, TQ])
    t["encT"] = din("encT", [C, T])
    # pre-tiled [ot, p, ct, o] layouts
    for k in ("wq1", "wk1", "wo1", "wq2", "wk2", "wo2"):
        t[k] = din(k, [C, C])
    t["wf1"] = din("wf1", [C, FF])        # pre-scaled by diag(g2), pretiled
    t["wf2"] = din("wf2", [FF, C])        # pretiled
    # natural [in, out] layouts (moving operand of the V matmul)
    t["wv1"] = din("wv1", [C, C])
    t["wv2"] = din("wv2", [C, C])
    t["cblk"] = din("cblk", [128, NCOL], F32)
    t["outT"] = nc.dram_tensor("outT", [C, TQ], F32, kind="ExternalOutput")

    with tile.TileContext(nc) as tc:
        for it in range(repeat):
            _emit(nc, tc, t, it)
    nc.compile()
    return nc



def _emit(nc, tc, t, it):
    from contextlib import ExitStack
    import concourse.bass as bass
    from concourse import mybir
    from concourse.tile import add_dep_helper

    F32 = mybir.dt.float32
    F32R = mybir.dt.float32r
    BF16 = mybir.dt.bfloat16
    AF = mybir.ActivationFunctionType
    ALU = mybir.AluOpType

    def w_ap(wdram, nctt, ot, a0, na):
        """pretiled arr[ot, p, ct, o]: [128, na, 128] view at (ot, a0)"""
        return bass.AP(tensor=wdram, offset=ot * nctt * 128 * 128 + a0 * 128,
                       ap=[[nctt * 128, 128], [128, na], [1, 128]])

    with ExitStack() as ctx:
        consts = ctx.enter_context(tc.tile_pool(name=f"con{it}", bufs=1))
        cb = consts.tile([128, NCOL], F32, tag="cb", name="cb")
        cb_dma = [None]

        def emit_cb(after=None):
            di = nc.scalar.dma_start(cb[:], t["cblk"][:])
            if after is not None:
                add_dep_helper(di.ins, after.ins,
                               reason="consts after critical startup stream")
            cb_dma[0] = di
        tri_sb = cb[:, COLS["tri"][0]:COLS["tri"][0] + 32].bitcast(BF16)
        ones128 = consts.tile([128, 128], BF16, tag="o128", name="o128")
        nc.vector.memset(ones128[:], 1.0)
        ones1 = consts.tile([1, 128], F32R, tag="o1", name="o1")
        nc.vector.memset(ones1[:].bitcast(F32), 1.0)
        eps_t = consts.tile([128, 1], F32, tag="eps", name="eps")
        nc.vector.memset(eps_t[:], 1e-5)

        lv = {k: cb[:, c0:c0 + n] for k, (c0, n) in COLS.items()}
        pad_sb = {"pad1": lv["pad1"], "pad2": lv["pad2"]}

        wpool = ctx.enter_context(tc.tile_pool(name=f"wp{it}", bufs=8))
        ypool = ctx.enter_context(tc.tile_pool(name=f"yp{it}", bufs=1))
        foldp = ctx.enter_context(tc.tile_pool(name=f"fp{it}", bufs=3))

        def fold_epilogue(ps, fold, ot, out_tile, func, bias_sb):
            """out = func(rstd*(ps - m*CS[ot]) + bias) given fold=(rstd, un)
            with un = -m*rstd, CS per-output-channel colsum."""
            rstd, un, cs = fold
            # PSUM reads and TensorScalarPtr are DVE/Act-only ops; GPSIMD
            # handles neither, so the whole fold chain stays on DVE.
            ftile = foldp.tile([128, TQ], F32, tag="ft", name="ft")
            nc.vector.tensor_mul(ftile[:], ps[:], rstd[:])
            nc.vector.scalar_tensor_tensor(
                out=ftile[:], in0=un[:], scalar=cs[:, ot:ot + 1], in1=ftile[:],
                op0=ALU.mult, op1=ALU.add)
            with nc.allow_low_precision(reason="bf16 activations"):
                nc.scalar.activation(out_tile[:], ftile[:], func,
                                     bias=bias_sb[:, ot:ot + 1])

        def new_y(dt=BF16):
            return [ypool.tile([128, TQ], dt, tag=f"y{i}", name=f"y{i}")
                    for i in range(NT)]

        def linear_T(wdram, cin, cout, in_tiles, n, epilogue, pp,
                     dma_out=None, first_cb=None):
            """psum[ot][:, q0:] = sum_ct W[ct,ot].T @ in[ct][:, q0:]"""
            nct = cin // 128
            for ot in range(cout // 128):
                wt = wpool.tile([128, nct, 128], BF16, tag="w", name="w")
                di = nc.sync.dma_start(wt[:], w_ap(wdram, nct, ot, 0, nct))
                if dma_out is not None:
                    dma_out.append(di)
                if ot == 0 and first_cb is not None:
                    first_cb()
                for q0 in range(0, n, 512):
                    ps = pp.tile([128, 512], F32, tag="pp", name="pp")
                    for ct in range(nct):
                        nc.tensor.matmul(ps[:], wt[:, ct, :],
                                         in_tiles[ct][:, q0:q0 + 512],
                                         start=(ct == 0), stop=(ct == nct - 1))
                    epilogue(ot, q0, ps)

        def ln_begin(sctx, tagp, lnps, lntag):
            sqp = sctx.enter_context(tc.tile_pool(name=f"sq{tagp}{it}", bufs=2))
            scr = sctx.enter_context(tc.tile_pool(name=f"ls{tagp}{it}", bufs=1))
            ps1 = lnps.tile([128, TQ], F32, tag=lntag, name="ln")
            ps2 = lnps.tile([128, TQ], F32, tag=lntag, name="ln")
            return {"sqp": sqp, "scr": scr, "ps1": ps1, "ps2": ps2}

        def ln_feed(st, y_tile, ct, sl=slice(0, TQ), sq_eng=None):
            """Accumulate sum(y) and sum(y^2) for one partition tile."""
            w = sl.stop - sl.start
            sq = st["sqp"].tile([128, TQ], BF16, tag="sq", name="sq")
            with nc.allow_low_precision(reason="bf16 squares ok"):
                (sq_eng or nc.vector).tensor_mul(sq[:, 0:w], y_tile[:, sl],
                                                 y_tile[:, sl])
            nc.tensor.matmul(st["ps1"][:, sl], ones128[:], y_tile[:, sl],
                             start=(ct == 0), stop=(ct == NT - 1),
                             skip_group_check=True)
            nc.tensor.matmul(st["ps2"][:, sl], ones128[:], sq[:, 0:w],
                             start=(ct == 0), stop=(ct == NT - 1),
                             skip_group_check=True)

        def ln_finish(st, y_in, g, b, out_t, statpool):
            """Stats chain now; the per-tile normalize (overwrites y_in) is
            returned as a closure so the caller can emit it where the vector
            engines are idle. Returns ((rstd, un), emit_norm)."""
            sp_ = statpool
            m = sp_.tile([128, TQ], F32, tag="m", name="m")
            nc.scalar.mul(m[:], st["ps1"][:], 1.0 / C)
            m2 = sp_.tile([128, TQ], F32, tag="v2", name="v2")
            nc.scalar.activation(m2[:], m[:], AF.Square)
            ms = sp_.tile([128, TQ], F32, tag="v", name="v")
            nc.vector.tensor_scalar_mul(ms[:], st["ps2"][:], 1.0 / C)
            nc.vector.tensor_sub(ms[:], ms[:], m2[:])
            # rstd = (v+eps)^-0.5 = exp(-0.5*ln(v+eps)): Ln+Exp live in the
            # same activation-table set as attention's Exp, so the Act engine
            # never reloads tables (Sqrt would force a 1.3us switch).
            nc.scalar.activation(ms[:], ms[:], AF.Ln, bias=eps_t[:])
            rstd = sp_.tile([128, TQ], F32, tag="r", name="r")
            nc.scalar.activation(rstd[:], ms[:], AF.Exp, scale=-0.5)
            un = sp_.tile([128, TQ], F32, tag="un", name="un")
            nc.vector.scalar_tensor_tensor(
                out=un[:], in0=m[:], scalar=-1.0, in1=rstd[:],
                op0=ALU.mult, op1=ALU.mult)

            def emit_norm(cts):
                for ct in cts:
                    eng = nc.gpsimd if ct in (3, 7) else nc.vector
                    d = foldp.tile([128, TQ], F32, tag="dn", name="dn",
                                   bufs=4)
                    eng.tensor_sub(d[:], y_in[ct][:], m[:])
                    eng.tensor_mul(d[:], d[:], rstd[:])
                    with nc.allow_low_precision(reason="bf16 activations"):
                        nc.scalar.activation(out_t[ct][:], d[:], AF.Identity,
                                             bias=b[:, ct:ct + 1],
                                             scale=g[:, ct:ct + 1])
            return (rstd, un), emit_norm

        def ln_chunk(st, y_in, g, b, y3t, sl):
            """Finish one column chunk: stats chain + normalize + out DMA."""
            scr = st["scr"]
            w = sl.stop - sl.start
            if "m" not in st:
                st["m"] = scr.tile([128, TQ], F32, tag="m", name="m", bufs=1)
                st["v"] = scr.tile([128, TQ], F32, tag="v", name="v", bufs=1)
                st["v2"] = scr.tile([128, TQ], F32, tag="v2", name="v2",
                                    bufs=1)
                st["r"] = scr.tile([128, TQ], F32, tag="r", name="r", bufs=1)
            m, ms, m2, rstd = st["m"], st["v"], st["v2"], st["r"]
            # m via Act (PSUM-read ok) in parallel with ms on DVE, then the
            # square also on Act back-to-back — one less cross-engine hop
            nc.scalar.mul(m[:, sl], st["ps1"][:, sl], 1.0 / C)
            nc.scalar.activation(m2[:, sl], m[:, sl], AF.Square)
            nc.vector.tensor_scalar_mul(ms[:, sl], st["ps2"][:, sl], 1.0 / C)
            nc.vector.tensor_sub(ms[:, sl], ms[:, sl], m2[:, sl])
            nc.scalar.activation(ms[:, sl], ms[:, sl], AF.Ln, bias=eps_t[:])
            nc.scalar.activation(rstd[:, sl], ms[:, sl], AF.Exp, scale=-0.5)
            last = True
            for ct in range(NT):
                eng = nc.vector if ct % 2 == 0 else nc.gpsimd
                d = scr.tile([128, w], F32, tag="dc", name="dc", bufs=6)
                eng.tensor_sub(d[:], y_in[ct][:, sl], m[:, sl])
                eng.tensor_mul(d[:], d[:], rstd[:, sl])
                nc.scalar.activation(y3t[:, ct, sl], d[:], AF.Identity,
                                     bias=b[:, ct:ct + 1],
                                     scale=g[:, ct:ct + 1])
                if last and ct in (1, 3, 5):
                    nc.sync.dma_start(
                        bass.AP(tensor=t["outT"],
                                offset=(ct - 1) * 128 * TQ + sl.start,
                                ap=[[TQ, 128], [128 * TQ, 2], [1, w]]),
                        y3t[:, ct - 1:ct + 1, sl])
            if last:
                nc.sync.dma_start(
                    bass.AP(tensor=t["outT"], offset=6 * 128 * TQ + sl.start,
                            ap=[[TQ, 128], [128 * TQ, 2], [1, w]]),
                    y3t[:, 6:8, sl])
            else:
                nc.sync.dma_start(
                    bass.AP(tensor=t["outT"], offset=sl.start,
                            ap=[[TQ, 128], [128 * TQ, NT], [1, w]]),
                    y3t[:, :, sl])

        def kv_stage(src, wk_d, wv_d, wq_d, q_src, padkey, kTt, vtt,
                     qTt, tagp, q_first, pp, qfold=None, src_dma=None,
                     after_k=None, mid_hook=None, q_first_cb=None):
            """Compute K_T, V (pad-masked, with the pad column in slot 64 for
            the softmax row sums), and Q_T from a resident transposed source.
            q_first=True: emit Q's matmuls before K (source already resident,
            weight DMAs returned through src_dma ordering); False: K first so
            the PE can run while the previous LN chain finishes."""
            with ExitStack() as kctx:
                def emit_q(dma_out=None):
                    if qfold is None:
                        def ep(ot, q0, ps):
                            with nc.allow_low_precision(reason="bf16 acts"):
                                nc.scalar.copy(qTt[ot][:], ps[:])
                        linear_T(wq_d, C, C, q_src, TQ, ep, pp,
                                 dma_out=dma_out, first_cb=q_first_cb)
                    else:
                        linear_T(wq_d, C, C, q_src, TQ,
                                 lambda ot, q0, ps: fold_epilogue(
                                     ps, qfold, ot, qTt[ot], AF.Identity,
                                     lv["bq2"]), pp, dma_out=dma_out)

                if q_first:
                    wd = []
                    emit_q(dma_out=wd)
                    if src_dma is not None:
                        src_dma(wd)

                def kep(ot, q0, ps):
                    # PSUM source: only DVE/Act may read PSUM (not GPSIMD)
                    with nc.allow_low_precision(reason="bf16 acts"):
                        nc.vector.tensor_copy(kTt[ot][:, q0:q0 + 512], ps[:])
                kd = []
                linear_T(wk_d, C, C, src, T, kep, pp, dma_out=kd)
                if after_k is not None:
                    after_k(kd)
                if not q_first:
                    emit_q()
                if mid_hook is not None:
                    mid_hook()
                wvp = kctx.enter_context(tc.tile_pool(name=f"wv{tagp}{it}",
                                                      bufs=2))
                for tt in range(KT):
                    with nc.allow_low_precision(reason="bf16 pad col"):
                        nc.gpsimd.tensor_scalar_mul(
                            vtt[tt][:, :, 64:65], ones128[:, 0:16],
                            pad_sb[padkey][:, tt:tt + 1])
                for half in range(2):
                    wvq = []
                    for cq in range(4):
                        wvt = wvp.tile([128, 2, 512], BF16, tag="wv",
                                       name="wv", bufs=8)
                        nc.sync.dma_start(
                            wvt[:],
                            bass.AP(tensor=wv_d,
                                    offset=512 * half + 256 * cq * C,
                                    ap=[[C, 128], [128 * C, 2], [1, 512]]))
                        wvq.append(wvt)
                    for tt in range(KT):
                        ps = pp.tile([128, 512], F32, tag="pp", name="pp")
                        for ct in range(NT):
                            nc.tensor.matmul(
                                ps[:], src[ct][:, 128 * tt:128 * (tt + 1)],
                                wvq[ct // 2][:, ct % 2, :],
                                start=(ct == 0), stop=(ct == NT - 1))
                        with nc.allow_low_precision(reason="bf16 acts"):
                            nc.vector.tensor_scalar_mul(
                                vtt[tt][:, 8 * half:8 * (half + 1), 0:64],
                                ps[:].rearrange("p (h d) -> p h d", d=64),
                                pad_sb[padkey][:, tt:tt + 1])

        def attention(qTt, kTt, vtt, wo_d, resid, g, b, out_t, causal,
                      sctx, tagp, scp, avp, rbp, op, statpool,
                      paired=False):
            """Scores/AV in transposed layout; O-projection output (+resid)
            is written back into the qT tiles (dead by then), then LN."""
            with ExitStack() as atx:
                pvp = atx.enter_context(tc.tile_pool(
                    name=f"pv{tagp}{it}", bufs=1))
                ppool = atx.enter_context(tc.tile_pool(
                    name=f"pt{tagp}{it}", bufs=8))
                sbp = atx.enter_context(tc.tile_pool(
                    name=f"sb{tagp}{it}", bufs=2))
                pv = [pvp.tile([128, TQ], BF16, tag=f"pv{i}", name=f"pv{i}")
                      for i in range(NT)]
                hseq = [(h0 + 4) % H for h0 in range(H)]
                for hpos, h in enumerate(hseq):
                    ct, off = h // 2, (h % 2) * 64
                    av = avp.tile([65, 512], F32, tag="av", name="av")
                    pend = None

                    def emit_av(p, c, k, av=av, h=h):
                        nc.tensor.matmul(av[:, c:512], vtt[k][:, h, 0:65],
                                         p[:, c:512],
                                         start=(k == 0), stop=(k == KT - 1))

                    if paired:
                        # score two key tiles into one 2-bank PSUM tile and
                        # exponentiate both with ONE Act instruction (Act
                        # per-instr overhead pins the attention windows).
                        # Causal pairs (k+4, k): the longer-range tile sits
                        # first so one exp range [64(k+4), 1024) covers both
                        # valid regions (the unwritten [512, 512+64k) sliver
                        # is exp'd but never read). AV for kt=0 is emitted
                        # first within its pair: its full-span matmul carries
                        # the PSUM-initializing start flag.
                        pairs = ([(k + 4, k) for k in range(4)] if causal
                                 else [(2 * pk, 2 * pk + 1)
                                       for pk in range(4)])

                        def emit_pair_avs(Pt2, ka, kb):
                            order = ((1, kb), (0, ka)) if causal                                 else ((0, ka), (1, kb))
                            for j, kt in order:
                                emit_av(Pt2[:, 512 * j:512 * (j + 1)],
                                        64 * kt if causal else 0, kt)

                        for ka, kb in pairs:
                            sp2 = scp.tile([128, 1024], F32, tag="sc",
                                           name="sc")
                            Pt2 = ppool.tile([128, 1024], BF16, tag="P",
                                             name="P")
                            for j, kt in ((0, ka), (1, kb)):
                                c0 = 64 * kt if causal else 0
                                nc.tensor.matmul(
                                    sp2[:, 512 * j + c0:512 * (j + 1)],
                                    kTt[ct][off:off + 64,
                                            128 * kt:128 * (kt + 1)],
                                    qTt[ct][off:off + 64, c0:512],
                                    start=True, stop=True)
                            e0 = 64 * ka if causal else 0
                            with nc.allow_low_precision(reason="bf16 probs"):
                                nc.scalar.activation(Pt2[:, e0:1024],
                                                     sp2[:, e0:1024],
                                                     AF.Exp, scale=0.125)
                            if causal:
                                for j, kt in ((0, ka), (1, kb)):
                                    d0 = 512 * j + 64 * kt
                                    nc.gpsimd.tensor_mul(
                                        Pt2[:, d0:d0 + 64],
                                        Pt2[:, d0:d0 + 64], tri_sb[:])
                            if pend is not None:
                                emit_pair_avs(*pend)
                            pend = (Pt2, ka, kb)
                        emit_pair_avs(*pend)
                    else:
                        for kt in range(KT):
                            c0 = 64 * kt if causal else 0
                            sp = scp.tile([128, 512], F32, tag="pp",
                                          name="pp")
                            Pt = ppool.tile([128, TQ], BF16, tag="P",
                                            name="P")
                            nc.tensor.matmul(
                                sp[:, c0:512],
                                kTt[ct][off:off + 64,
                                        128 * kt:128 * (kt + 1)],
                                qTt[ct][off:off + 64, c0:512],
                                start=True, stop=True)
                            with nc.allow_low_precision(reason="bf16 probs"):
                                nc.scalar.activation(Pt[:, c0:512],
                                                     sp[:, c0:512],
                                                     AF.Exp, scale=0.125)
                            if causal:
                                nc.gpsimd.tensor_mul(
                                    Pt[:, 64 * kt:64 * (kt + 1)],
                                    Pt[:, 64 * kt:64 * (kt + 1)], tri_sb[:])
                            if pend is not None:
                                emit_av(*pend)
                            pend = (Pt, c0, kt)
                        emit_av(*pend)
                    rinv = sbp.tile([1, 512], F32R, tag="ri", name="ri")
                    with nc.allow_low_precision(reason="fp32r rounding ok"):
                        nc.vector.reciprocal(rinv[:], av[64:65, :])
                    if rbp is not None:
                        rb_ps = rbp.tile([128, 512], F32, tag="rb",
                                         name="rb")
                    else:
                        rb_ps = op.tile([128, 512], F32, tag="o", name="o")
                    nc.tensor.matmul(rb_ps[:], ones1[:], rinv[:],
                                     start=True, stop=True)
                    if hpos >= H - 2:
                        # tail heads: lift AV out of PSUM on the (now idle)
                        # Act engine in parallel with the reciprocal, and
                        # multiply against the PSUM broadcast directly — one
                        # DVE hop shorter, so WO isn't held up.
                        avs = sbp.tile([64, 512], F32, tag="avs", name="avs")
                        nc.scalar.copy(avs[:], av[0:64, :])
                        with nc.allow_low_precision(reason="bf16 acts"):
                            nc.vector.tensor_mul(pv[ct][off:off + 64, :],
                                                 avs[:], rb_ps[0:64, :])
                    else:
                        rb = sbp.tile([64, 512], F32, tag="rs", name="rs")
                        nc.vector.tensor_copy(rb[:], rb_ps[0:64, :])
                        with nc.allow_low_precision(reason="bf16 acts"):
                            nc.vector.tensor_mul(pv[ct][off:off + 64, :],
                                                 av[0:64, :], rb[:])
                lnst = ln_begin(atx, tagp, avp, "av")
                for co in range(NT):
                    wt = wpool.tile([128, NT, 128], BF16, tag="w", name="w")
                    nc.sync.dma_start(wt[:], w_ap(wo_d, NT, co, 0, NT))
                    ps = op.tile([128, 512], F32, tag="o", name="o")
                    for i, cc in enumerate((c + 2) % NT for c in range(NT)):
                        nc.tensor.matmul(ps[:], wt[:, cc, :], pv[cc][:],
                                         start=(i == 0), stop=(i == NT - 1))
                    with nc.allow_low_precision(reason="bf16 acts"):
                        nc.vector.tensor_add(qTt[co][:], ps[:], resid[co][:])
                    ln_feed(lnst, qTt[co], co)
                return ln_finish(lnst, qTt, g, b, out_t, statpool)

        # ================= main flow =================
        qpool = ctx.enter_context(tc.tile_pool(name=f"qp{it}", bufs=1))
        statp = ctx.enter_context(tc.tile_pool(name=f"st{it}", bufs=1))

        def new_q(pfx="q"):
            return [qpool.tile([128, TQ], BF16, tag=f"{pfx}{i}",
                               name=f"{pfx}{i}") for i in range(NT)]

        with ExitStack() as actx:
            qkv = actx.enter_context(tc.tile_pool(name=f"qkv{it}", bufs=1))
            srcp = actx.enter_context(tc.tile_pool(name=f"sr{it}", bufs=1))
            p1 = actx.enter_context(ExitStack())
            pps = p1.enter_context(tc.tile_pool(
                name=f"ps{it}", bufs=3, space="PSUM"))
            scs = p1.enter_context(tc.tile_pool(
                name=f"ss{it}", bufs=3, space="PSUM"))
            avps = p1.enter_context(tc.tile_pool(
                name=f"as{it}", bufs=2, space="PSUM"))

            def new_kv():
                k = [qkv.tile([128, T], BF16, tag=f"k{i}", name=f"k{i}")
                     for i in range(NT)]
                v = [qkv.tile([128, 16, 65], BF16, tag=f"v{i}", name=f"v{i}")
                     for i in range(KT)]
                return k, v

            def load_src(dram, tag, eng, after=None):
                st_ = srcp.tile([128, NT, T], BF16, tag=tag, name=tag)
                dmas = []
                for th in range(2):
                    di = eng.dma_start(
                        st_[:, :, 512 * th:512 * (th + 1)],
                        bass.AP(tensor=dram, offset=512 * th,
                                ap=[[T, 128], [128 * T, NT], [1, 512]]))
                    if after is not None and th < len(after):
                        add_dep_helper(di.ins, after[th].ins,
                                       reason="src after critical weights")
                    dmas.append(di)
                return [st_[:, i, :] for i in range(NT)], dmas

            # ---- self-attention + AddNorm ----
            qT = new_q()
            kTt, vtt = new_kv()
            with ExitStack() as sctx:
                xt_t = qpool.tile([128, NT, TQ], BF16, tag="xt", name="xt")
                xTl_sb = [xt_t[:, i, :] for i in range(NT)]

                def xt_emit():
                    # emitted after the first wq1 tile DMA so the weight
                    # stream heads the startup DMA queue; first ct alone so
                    # the very first matmul can fire early
                    dis = []
                    for c0_, nc_ in ((0, 1), (1, 3), (4, 4)):
                        dis.append(nc.sync.dma_start(
                            xt_t[:, c0_:c0_ + nc_, :],
                            bass.AP(tensor=t["xTl"], offset=c0_ * 128 * TQ,
                                    ap=[[TQ, 128], [128 * TQ, nc_], [1, TQ]])))
                    emit_cb(after=dis[0])
                src1, src2 = [None], [None]

                def src_dma(wd):
                    src1[0] = load_src(t["xT"], "sA", nc.gpsimd,
                                       after=[wd[1], wd[3]])[0]

                def enc_dma(kd):
                    # prefetch encT on the gpsimd queue once the K-weight
                    # stream (critical for the current stage) is in flight
                    src2[0] = load_src(t["encT"], "sB", nc.gpsimd,
                                       after=[kd[3], kd[7]])[0]

                class _SrcProxy:
                    def __getitem__(self, i):
                        return src1[0][i]
                kv_stage(_SrcProxy(), t["wk1"], t["wv1"], t["wq1"], xTl_sb,
                         "pad1", kTt, vtt, qT, "s", True, pps,
                         src_dma=src_dma, after_k=enc_dma,
                         q_first_cb=xt_emit)
                y1 = new_y()
                fold1, norm1 = attention(qT, kTt, vtt, t["wo1"], xTl_sb,
                                         lv["g1"], lv["b1"], y1, True,
                                         sctx, "s", bigp, avps, None, ops,
                                         statpool=statp, paired=True)

            # ---- cross-attention + AddNorm (fresh tile generations) ----
            qT2 = new_q("x")
            kTt2, vtt2 = new_kv()
            with ExitStack() as cctx:
                kv_stage(src2[0], t["wk2"], t["wv2"], t["wq2"], qT,
                         "pad2", kTt2, vtt2, qT2, "c", False, pps,
                         qfold=(fold1[0], fold1[1], lv["csq2"]),
                         mid_hook=lambda: norm1(range(NT)))
                # release the self-stage PSUM pools so cross-attention can
                # afford 2-bank paired score tiles (no PE filler competes
                # for PSUM in this window)
                p1.close()
                sc2ps = cctx.enter_context(tc.tile_pool(
                    name=f"sc{it}", bufs=2, space="PSUM"))
                avps2 = cctx.enter_context(tc.tile_pool(
                    name=f"a2{it}", bufs=2, space="PSUM"))
                ops2 = cctx.enter_context(tc.tile_pool(
                    name=f"o2{it}", bufs=2, space="PSUM"))
                y2 = new_y()
                fold2, norm2 = attention(qT2, kTt2, vtt2, t["wo2"], y1,
                                         lv["g2"], lv["b2"], y2, False,
                                         cctx, "c", sc2ps, avps2, None,
                                         ops2, statpool=statp, paired=True)

        # ---- FFN + AddNorm ----
        with ExitStack() as fctx:
            y3p = fctx.enter_context(tc.tile_pool(name=f"y3{it}", bufs=1))
            y3t = y3p.tile([128, NT, TQ], F32, tag="z", name="z")
            lnps3 = fctx.enter_context(tc.tile_pool(
                name=f"l3{it}", bufs=2, space="PSUM"))
            lnst3 = ln_begin(fctx, "f", lnps3, "ln")
            ffold = (fold2[0], fold2[1], lv["csf1"])
            with ExitStack() as mctx:
                hp = mctx.enter_context(tc.tile_pool(name=f"hp{it}", bufs=1))
                w1p = mctx.enter_context(tc.tile_pool(name=f"w1{it}", bufs=6))
                w2p = mctx.enter_context(tc.tile_pool(name=f"w2{it}", bufs=2))
                pp1 = mctx.enter_context(tc.tile_pool(
                    name=f"f1{it}", bufs=3, space="PSUM"))
                pp2 = mctx.enter_context(tc.tile_pool(
                    name=f"f2{it}", bufs=3, space="PSUM"))
                NF = 16
                # fb=0: FFN1 first half; FFN2 partials into y3t
                h_sb = [hp.tile([128, TQ], BF16, tag=f"h{i}",
                                name=f"h{i}") for i in range(NF)]
                for f in range(NF):
                    w1t = w1p.tile([128, NT, 128], BF16, tag="w1", name="w1")
                    nc.sync.dma_start(w1t[:], w_ap(t["wf1"], NT, f, 0, NT))
                    ps = pp1.tile([128, 512], F32, tag="p1", name="p1")
                    for ct in range(NT):
                        nc.tensor.matmul(ps[:], w1t[:, ct, :], qT2[ct][:],
                                         start=(ct == 0), stop=(ct == NT - 1))
                    fold_epilogue(ps, ffold, f, h_sb[f], AF.Relu, lv["bf1"])
                for co in range(NT):
                    w2t = w2p.tile([128, NF, 128], BF16, tag="w2", name="w2")
                    nc.sync.dma_start(w2t[:], w_ap(t["wf2"], FT, co, 0, NF))
                    ps = pp2.tile([128, 512], F32, tag="p2", name="p2")
                    for f in range(NF):
                        nc.tensor.matmul(ps[:], w2t[:, f, :], h_sb[f][:],
                                         start=(f == 0), stop=(f == NF - 1))
                    nc.vector.tensor_copy(y3t[:, co, :], ps[:])
                    norm2([co])  # LN2 normalize of y2 rides the fb0 window
                # fb=1: FFN1 second half; FFN2 in column chunks so each
                # chunk's LN3 chain + output DMA overlaps the next chunk's
                # matmuls on the PE.
                h_sb = [hp.tile([128, TQ], BF16, tag=f"h{i}",
                                name=f"h{i}") for i in range(NF)]
                w2h = []
                for f in range(NF):
                    fg = NF + f
                    w1t = w1p.tile([128, NT, 128], BF16, tag="w1", name="w1")
                    nc.sync.dma_start(w1t[:], w_ap(t["wf1"], NT, fg, 0, NT))
                    ps = pp1.tile([128, 512], F32, tag="p1", name="p1")
                    for ct in range(NT):
                        nc.tensor.matmul(ps[:], w1t[:, ct, :], qT2[ct][:],
                                         start=(ct == 0), stop=(ct == NT - 1))
                    fold_epilogue(ps, ffold, fg, h_sb[f], AF.Relu, lv["bf1"])
                    if f % 2 == 0:
                        co = f // 2
                        w2t = w2p.tile([128, NF, 128], BF16, tag=f"wc{co}",
                                       name=f"wc{co}", bufs=1)
                        nc.sync.dma_start(
                            w2t[:], w_ap(t["wf2"], FT, co, NF, NF))
                        w2h.append(w2t)
                for ca, cbnd in ((0, 288), (288, 512)):
                    sl = slice(ca, cbnd)
                    wch = cbnd - ca
                    for co in range(NT):
                        psf = pp2.tile([128, 512], F32, tag="p2", name="p2")
                        ps = psf[:, 0:wch]
                        for f in range(NF):
                            nc.tensor.matmul(ps[:], w2h[co][:, f, :],
                                             h_sb[f][:, sl],
                                             start=(f == 0),
                                             stop=(f == NF - 1))
                        nc.vector.scalar_tensor_tensor(
                            out=y3t[:, co, sl], in0=ps[:],
                            scalar=lv["bf2"][:, co:co + 1],
                            in1=y3t[:, co, sl],
                            op0=ALU.add, op1=ALU.add)
                        with nc.allow_low_precision(reason="bf16 residual"):
                            nc.vector.tensor_add(y2[co][:, sl],
                                                 y3t[:, co, sl],
                                                 y2[co][:, sl])
                        ln_feed(lnst3, y2[co], co, sl)
                    ln_chunk(lnst3, y2, lv["g3"], lv["b3"], y3t, sl)


def _shard(inputs):
    import ml_dtypes
    BF = ml_dtypes.bfloat16
    x = np.asarray(inputs["x"], dtype=np.float32)
    enc = np.asarray(inputs["enc_out"], dtype=np.float32)
    tpad = np.asarray(inputs["tgt_pad_mask"]).astype(np.float32)
    spad = np.asarray(inputs["src_pad_mask"]).astype(np.float32)
    ws = {k: np.asarray(inputs[k], dtype=np.float32)
          for k in ("Wq1", "Wk1", "Wv1", "Wo1", "Wq2", "Wk2", "Wv2", "Wo2",
                    "Wf1", "Wf2")}
    lnv = {k: np.asarray(inputs[k], dtype=np.float32)
           for k in ("ln1_g", "ln1_b", "ln2_g", "ln2_b", "ln3_g", "ln3_b",
                     "bf1", "bf2")}

    def pret(W):  # [cin, cout] -> [ot, p, ct, o] pretiled bf16
        cin, cout = W.shape
        return np.ascontiguousarray(
            W.reshape(cin // 128, 128, cout // 128, 128)
            .transpose(2, 1, 0, 3).astype(BF))

    # LN1 affine folded through Wq2; LN2 affine folded through Wf1.
    wq2f = lnv["ln1_g"][:, None] * ws["Wq2"]
    csq2 = wq2f.astype(BF).astype(np.float32).sum(axis=0)
    bq2 = lnv["ln1_b"] @ ws["Wq2"]
    wf1f = lnv["ln2_g"][:, None] * ws["Wf1"]
    csf1 = wf1f.astype(BF).astype(np.float32).sum(axis=0)
    bf1f = lnv["bf1"] + lnv["ln2_b"] @ ws["Wf1"]

    wt = {"wq1": pret(ws["Wq1"]), "wk1": pret(ws["Wk1"]),
          "wo1": pret(ws["Wo1"]), "wq2": pret(wq2f), "wk2": pret(ws["Wk2"]),
          "wo2": pret(ws["Wo2"]), "wf1": pret(wf1f), "wf2": pret(ws["Wf2"]),
          "wv1": np.ascontiguousarray(ws["Wv1"].astype(BF)),
          "wv2": np.ascontiguousarray(ws["Wv2"].astype(BF))}

    def cols(v):  # length n -> [128, n//128]
        return np.asarray(v, np.float32).reshape(-1, 128).T

    cblk = np.zeros((128, NCOL), np.float32)
    for k, vec in (("pad1", 1.0 - tpad[0]), ("pad2", 1.0 - spad[0]),
                   ("g1", lnv["ln1_g"]), ("b1", lnv["ln1_b"]),
                   ("g2", lnv["ln2_g"]), ("b2", lnv["ln2_b"]),
                   ("g3", lnv["ln3_g"]), ("b3", lnv["ln3_b"]),
                   ("bf1", bf1f), ("csq2", csq2), ("bq2", bq2),
                   ("csf1", csf1), ("bf2", lnv["bf2"])):
        c0, n = COLS[k]
        if k not in ("pad1", "pad2"):
            cblk[:, c0:c0 + n] = cols(vec)

    in_maps = []
    for b in range(B):
        xTb = np.ascontiguousarray(x[b].T.astype(BF))
        eTb = np.ascontiguousarray(enc[b].T.astype(BF))
        p1v, p2v = 1.0 - tpad[b], 1.0 - spad[b]
        for h in range(2):
            xTlb = np.ascontiguousarray(x[b, h::2, :].T.astype(BF))
            trih = np.ascontiguousarray(
                (np.arange(128)[:, None] <= 2 * np.arange(64)[None, :] + h
                 ).astype(BF))
            cb = cblk.copy()
            cb[:, 0:8] = cols(p1v)
            cb[:, 8:16] = cols(p2v)
            cb[:, 152:184] = trih.view(np.float32)
            m = {"xT": xTb, "xTl": xTlb, "encT": eTb,
                 "cblk": np.ascontiguousarray(cb)}
            m.update(wt)
            in_maps.append(m)
    return in_maps


def _get_nc(repeat=1):
    if repeat not in _CACHE:
        _CACHE[repeat] = _build(repeat)
    return _CACHE[repeat]


def kernel(**inputs):
    from concourse.bass_utils import run_bass_kernel_spmd
    nc = _get_nc()
    in_maps = _shard(inputs)
    res = run_bass_kernel_spmd(nc, in_maps, core_ids=list(range(8)))
    out = np.empty((B, T, C), np.float32)
    for core in range(8):
        b, h = core // 2, core % 2
        out[b, h::2, :] = res.results[core]["outT"].T
    return out
